# revision 2
# baseline (speedup 1.0000x reference)
"""Trainium2 Bass kernel for nn_Attention_84473416778449.

Reference computation (B=2, S=2048, D=1024, H=16, HD=64, fp32):
    q/k/v = x @ w{q,k,v}.T ; RoPE(q, k) ; causal softmax attention ; out @ wo.T

Sharding: 8 cores = (batch 2) x (head-group 4). Each core computes 4 heads of
one batch end-to-end and a partial output projection over its 256 channels;
the host sums the 4 partials per batch.

v2 structure (evidence-driven from the v1 trace):
  - All matmul operands are converted to fp16 on the HOST; DRAM inputs and the
    output are fp16 (cos/sin stay f32). This removes the in-kernel conversion
    copies and the 19us cold-start DMA wall of v1.
  - The PE is warmed with dummy matmuls while the first DMAs land, so the HAM
    clock gate is at 8/8 when real work starts (v1 paid ~15us at 4/8).
  - Score matmuls for the two heads of a qT/kT tile are issued back-to-back to
    disjoint PE row groups (tile_position (0,0)/(64,0)) so they execute
    concurrently, halving score matmul occupancy.
  - The attention loop is software-pipelined: PV(kt-1) is emitted after
    scores(kt), and projection/output-projection work for the next block is
    interleaved into the attention steps as "filler" so the PE never waits on
    the exp (ACT) latency.

Layout (per core):
    xT    [1024, 2048] fp16 (host pre-transposed x[b].T), 512-col slices
    wq/wk/wv staged [128, 8*256] fp16 (host pre-laid-out), woT [256,1024] fp16
    qT/kT [2 x (128, 2048)] two heads per tile (dh on partitions)
    v     [16 x (128, 4*65+64)] s on partitions; per-head 64 cols + ones col
          (the ones column makes the PV matmul emit softmax denominators in
          psum row 64; the PV weight slice is padded to 128 cols)
    scores^T tiles [sk=128, sq<=512] -> exp on ACT -> causal affine_select ->
          PV accumulate -> normalize via broadcast + approx reciprocal.
Diagonal score tiles are trimmed to their unmasked column range; fully
masked tiles are skipped entirely.
"""
import sys

if "/opt/trn_rl_repo" not in sys.path:
    sys.path.insert(0, "/opt/trn_rl_repo")

import numpy as np

import concourse.bass as bass
import concourse.mybir as mybir
import concourse.tile as tile
from concourse import bacc
from concourse.bass_utils import run_bass_kernel_spmd

B, S, D, H, HD = 2, 2048, 1024, 16, 64
NCORES = 8
GROUPS = 4            # head groups
GH = H // GROUPS      # heads per group = 4
GC = GH * HD          # channels per group = 256
KT = D // 128         # 8 k-tiles over D
ST = S // 128         # 16 s-tiles
QB = 4                # sq blocks of 512
QW = S // QB          # 512
VW = GH * (HD + 1)    # 260: v tile payload columns

f32 = mybir.dt.float32
MMDT = mybir.dt.float16   # matmul-operand dtype
Exp = mybir.ActivationFunctionType.Exp
Copy = mybir.ActivationFunctionType.Copy

_cache = {}


def _build():
    nc = bacc.Bacc("TRN2", num_devices=NCORES)

    xT = nc.dram_tensor("xT", [D, S], MMDT, kind="ExternalInput").ap()
    # host-packed: [p, kt*GC + c] = w.T[kt*128 + p, c]
    wqs = nc.dram_tensor("wqs", [128, KT * GC], MMDT, kind="ExternalInput").ap()
    wks = nc.dram_tensor("wks", [128, KT * GC], MMDT, kind="ExternalInput").ap()
    wvs = nc.dram_tensor("wvs", [128, KT * GC], MMDT, kind="ExternalInput").ap()
    woT = nc.dram_tensor("woT", [GC, D], MMDT, kind="ExternalInput").ap()
    cs2 = nc.dram_tensor("cs2", [128, S], f32, kind="ExternalInput").ap()
    sn2 = nc.dram_tensor("sn2", [128, S], f32, kind="ExternalInput").ap()
    out = nc.dram_tensor("out", [S, D], MMDT, kind="ExternalOutput").ap()

    with tile.TileContext(nc) as tc:
        with tc.tile_pool(name="persist", bufs=1) as pp, \
             tc.tile_pool(name="rope", bufs=3) as rp, \
             tc.tile_pool(name="probs", bufs=6) as wp, \
             tc.tile_pool(name="outsb", bufs=3) as op_, \
             tc.tile_pool(name="small", bufs=3) as sp:

            # ---- PE warm-up: dummy matmuls while the first DMAs land ------
            warm = pp.tile([128, 128], MMDT, tag="warm")
            nc.gpsimd.memset(warm[:], 0.0)
            with tc.tile_pool(name="psW", bufs=1, space="PSUM") as psW:
                wps = psW.tile([128, 128], f32, tag="wps")
                for _ in range(32):
                    nc.tensor.matmul(wps[:], warm[:], warm[:],
                                     start=True, stop=True)

            # ---- first-needed loads go out before anything else ------------
            xTr = [[pp.tile([128, QW], MMDT, tag=f"xT{kt}_{cb}",
                            name=f"xTr{kt}_{cb}") for cb in range(QB)]
                   for kt in range(KT)]

            def load_x_block(cb, eng):
                for kt in range(KT):
                    eng.dma_start(
                        xTr[kt][cb][:],
                        xT[kt * 128:(kt + 1) * 128, cb * QW:(cb + 1) * QW])

            def load_wT(src):
                t = pp.tile([128, KT * GC], MMDT, tag=f"w{src.tensor.name}",
                            name=f"w{src.tensor.name}")
                nc.gpsimd.dma_start(t[:], src)
                return t

            wq_s = load_wT(wqs)
            load_x_block(0, nc.sync)
            cs_sb = pp.tile([128, S], f32, tag="cs")
            nc.sync.dma_start(cs_sb[:], cs2[:])
            sn_sb = pp.tile([128, S], f32, tag="sn")
            nc.sync.dma_start(sn_sb[:], sn2[:])
            load_x_block(1, nc.gpsimd)

            # ---- constants (built in fp32, cast via DVE copy) --------------
            cscratch = pp.tile([128, 128], f32, tag="cscratch")
            nc.gpsimd.memset(cscratch[:], 0.0)
            for blk in range(2):
                sub = cscratch[blk * 64:(blk + 1) * 64,
                               blk * 64:(blk + 1) * 64]
                nc.gpsimd.affine_select(   # -1 where p - f == 32
                    out=sub, in_=sub, pattern=[[-1, 64]], base=-32,
                    channel_multiplier=1,
                    compare_op=mybir.AluOpType.not_equal, fill=-1.0)
                nc.gpsimd.affine_select(   # +1 where f - p == 32
                    out=sub, in_=sub, pattern=[[1, 64]], base=-32,
                    channel_multiplier=-1,
                    compare_op=mybir.AluOpType.not_equal, fill=1.0)
            rt2 = pp.tile([128, 128], MMDT, tag="rt2")
            nc.vector.tensor_copy(rt2[:], cscratch[:])
            ones_f = pp.tile([128, GH], f32, tag="ones_f")
            nc.gpsimd.memset(ones_f[:], 1.0)
            zeros_f = pp.tile([128, 64], f32, tag="zeros_f")
            nc.gpsimd.memset(zeros_f[:], 0.0)

            # ---- remaining loads -------------------------------------------
            wk_s, wv_s = load_wT(wks), load_wT(wvs)
            wo_s = []
            for kt in range(2):
                t = pp.tile([128, D], MMDT, tag=f"wo{kt}", name=f"wo{kt}")
                nc.gpsimd.dma_start(t[:], woT[kt * 128:(kt + 1) * 128, :])
                wo_s.append(t)

            qT = [[pp.tile([128, QW], MMDT, tag=f"qT{i}_{b}",
                           name=f"qT{i}_{b}") for b in range(QB)]
                  for i in range(2)]
            kTt = [[pp.tile([128, QW], MMDT, tag=f"kT{i}_{b}",
                            name=f"kT{i}_{b}") for b in range(QB)]
                   for i in range(2)]
            attnT = [[pp.tile([128, QW], MMDT, tag=f"aT{i}_{b}",
                              name=f"aT{i}_{b}") for b in range(QB)]
                     for i in range(2)]
            v_sb = [pp.tile([128, VW + 64], MMDT, tag=f"v{i}",
                            name=f"v{i}") for i in range(ST)]

            with tc.tile_pool(name="psA", bufs=2, space="PSUM") as psA, \
                 tc.tile_pool(name="psR", bufs=1, space="PSUM") as psR, \
                 tc.tile_pool(name="psS", bufs=3, space="PSUM") as psS, \
                 tc.tile_pool(name="psO", bufs=2, space="PSUM") as psO:

                def qk_chain_chunks(w_src, dst, hp, sb):
                    cols = slice(sb * QW, (sb + 1) * QW)
                    box = {}

                    def mk_mm(kt):
                        def f():
                            if kt == 0:
                                box['pq'] = psA.tile(
                                    [128, QW], f32, tag="proj",
                                    name=f"pq{hp}_{sb}")
                            nc.tensor.matmul(
                                box['pq'][:],
                                w_src[:, kt * GC + hp * 128:
                                      kt * GC + hp * 128 + 128],
                                xTr[kt][sb][:],
                                start=(kt == 0), stop=(kt == KT - 1))
                        return f

                    def fin():
                        pq = box['pq']
                        tcs = rp.tile([128, QW], MMDT, tag="tcs")
                        nc.vector.tensor_tensor(
                            out=tcs[:], in0=pq[:], in1=cs_sb[:, cols],
                            op=mybir.AluOpType.mult)
                        tsn = rp.tile([128, QW], MMDT, tag="tsn")
                        nc.vector.tensor_tensor(
                            out=tsn[:], in0=pq[:], in1=sn_sb[:, cols],
                            op=mybir.AluOpType.mult)
                        pr = psR.tile([128, QW], f32, tag="rope")
                        nc.tensor.matmul(pr[:], rt2[:], tsn[:],
                                         start=True, stop=True)
                        nc.vector.tensor_tensor(
                            out=dst[hp][sb][:], in0=pr[:], in1=tcs[:],
                            op=mybir.AluOpType.add)
                    return [mk_mm(kt) for kt in range(KT)] + [fin]

                def v_chain_chunks(st):
                    box = {}

                    def mk_mm(kt):
                        def f():
                            if kt == 0:
                                box['pv'] = psA.tile(
                                    [128, QW], f32, tag="proj",
                                    name=f"pv_{st}")
                            nc.tensor.matmul(
                                box['pv'][:, 0:GC],
                                xTr[kt][st // 4][:, (st % 4) * 128:
                                                 (st % 4) * 128 + 128],
                                wv_s[:, kt * GC:(kt + 1) * GC],
                                start=(kt == 0), stop=(kt == KT - 1))
                        return f

                    def fin():
                        pv = box['pv']
                        vt = v_sb[st]
                        vhe = vt[:, 0:VW].rearrange(
                            "p (h e) -> p h e", e=HD + 1)
                        nc.vector.tensor_copy(vt[:, VW:VW + 64], zeros_f[:])
                        nc.vector.tensor_copy(vhe[:, :, HD:HD + 1], ones_f[:])
                        nc.scalar.activation(
                            vhe[:, :, 0:HD],
                            pv[:, 0:GC].rearrange("p (h d) -> p h d", d=HD),
                            Copy)
                    return [mk_mm(kt) for kt in range(KT)] + [fin]

                def emit_out(st, db):
                    pc = psA.tile([128, QW], f32, tag="proj",
                                  name=f"pc_{st}_{db}")
                    for kt in range(2):
                        nc.tensor.matmul(
                            pc[:],
                            attnT[kt][st // 4][:, (st % 4) * 128:
                                               (st % 4) * 128 + 128],
                            wo_s[kt][:, db * QW:(db + 1) * QW],
                            start=(kt == 0), stop=(kt == 1))
                    ob = op_.tile([128, QW], MMDT, tag="outsb")
                    nc.vector.tensor_copy(ob[:], pc[:])
                    nc.sync.dma_start(
                        out[st * 128:(st + 1) * 128,
                            db * QW:(db + 1) * QW], ob[:])

                def proj_chunks(sbx):
                    chunks = []
                    for w_src, dst in ((wq_s, qT), (wk_s, kTt)):
                        for hp in range(2):
                            chunks += qk_chain_chunks(w_src, dst, hp, sbx)
                    for st in range(sbx * 4, sbx * 4 + 4):
                        chunks += v_chain_chunks(st)
                    return chunks

                def emit_attention(qb, fillers):
                    nsk = (qb + 1) * 4
                    steps_total = 2 * nsk
                    step = 0
                    fi = 0
                    for hp in range(2):
                        hA, hB = 2 * hp, 2 * hp + 1
                        poA = psO.tile([128, QW], f32, tag="pvacc",
                                       name=f"poA_{hp}_{qb}")
                        poB = psO.tile([128, QW], f32, tag="pvacc",
                                       name=f"poB_{hp}_{qb}")
                        prev = None

                        def pv_pair(p):
                            kt, c0, cw, prtA, prtB = p
                            nc.tensor.matmul(
                                poA[:, c0:QW],
                                v_sb[kt][:, hA * (HD + 1):
                                         hA * (HD + 1) + 128],
                                prtA[:, 0:cw],
                                start=(kt == 0), stop=(kt == nsk - 1))
                            nc.tensor.matmul(
                                poB[:, c0:QW],
                                v_sb[kt][:, hB * (HD + 1):
                                         hB * (HD + 1) + 128],
                                prtB[:, 0:cw],
                                start=(kt == 0), stop=(kt == nsk - 1))

                        for kt in range(nsk):
                            c0 = max(0, kt * 128 - qb * QW)
                            cw = QW - c0
                            psa = psS.tile([128, QW], f32, tag="score",
                                           name=f"scA_{hp}_{qb}_{kt}")
                            psb = psS.tile([128, QW], f32, tag="score",
                                           name=f"scB_{hp}_{qb}_{kt}")
                            nc.tensor.matmul(
                                psa[:, 0:cw],
                                kTt[hp][kt // 4][0:64,
                                                 (kt % 4) * 128:
                                                 (kt % 4) * 128 + 128],
                                qT[hp][qb][0:64, c0:QW],
                                start=True, stop=True,
                                tile_position=(0, 0))
                            nc.tensor.matmul(
                                psb[:, 0:cw],
                                kTt[hp][kt // 4][64:128,
                                                 (kt % 4) * 128:
                                                 (kt % 4) * 128 + 128],
                                qT[hp][qb][64:128, c0:QW],
                                start=True, stop=True,
                                tile_position=(64, 0))
                            prtA = wp.tile([128, QW], MMDT, tag="probs",
                                           name=f"prA_{hp}_{qb}_{kt}")
                            prtB = wp.tile([128, QW], MMDT, tag="probs",
                                           name=f"prB_{hp}_{qb}_{kt}")
                            nc.scalar.activation(
                                prtA[:, 0:cw], psa[:, 0:cw], Exp, scale=0.125)
                            nc.scalar.activation(
                                prtB[:, 0:cw], psb[:, 0:cw], Exp, scale=0.125)
                            if kt >= nsk - 4:
                                for prt in (prtA, prtB):
                                    nc.gpsimd.affine_select(
                                        out=prt[:, 0:cw], in_=prt[:, 0:cw],
                                        pattern=[[1, cw]], base=0,
                                        channel_multiplier=-1,
                                        compare_op=mybir.AluOpType.is_ge,
                                        fill=0.0)
                            # filler work covers the exp latency before the
                            # pipelined PV of the previous step
                            step += 1
                            want = (len(fillers) * step) // steps_total
                            while fi < want:
                                fillers[fi]()
                                fi += 1
                            if prev is not None:
                                pv_pair(prev)
                            prev = (kt, c0, cw, prtA, prtB)
                        pv_pair(prev)
                        for h, po in ((hA, poA), (hB, poB)):
                            d0 = sp.tile([1, QW], f32, tag="den0")
                            nc.vector.tensor_copy(d0[:], po[HD:HD + 1, :])
                            dn = sp.tile([64, QW], f32, tag="denb")
                            nc.gpsimd.partition_broadcast(dn[:], d0[:])
                            rcb = sp.tile([64, QW], f32, tag="recb")
                            scr = sp.tile([64, QW], f32, tag="scrb")
                            nc.vector.reciprocal_approx_accurate(
                                out=rcb[:], in_=dn[:], scratch=scr[:])
                            nc.vector.tensor_tensor(
                                out=attnT[hp][qb][(h % 2) * 64:
                                                  (h % 2) * 64 + 64, :],
                                in0=po[0:HD, :], in1=rcb[:],
                                op=mybir.AluOpType.mult)
                    while fi < len(fillers):
                        fillers[fi]()
                        fi += 1

                # sb=0 projections are a dense block (nothing to hide behind)
                for ch in proj_chunks(0):
                    ch()
                for sb in range(QB):
                    if sb + 2 < QB:
                        load_x_block(sb + 2, nc.gpsimd)
                    fillers = []
                    if sb > 0:
                        fillers += [
                            (lambda st=st, db=db: emit_out(st, db))
                            for st in range((sb - 1) * 4, sb * 4)
                            for db in range(2)]
                    if sb + 1 < QB:
                        fillers += proj_chunks(sb + 1)
                    emit_attention(sb, fillers)
                for st in range((QB - 1) * 4, QB * 4):
                    for db in range(2):
                        emit_out(st, db)

    nc.compile()
    return nc


def _shard_inputs(x, cos, sin, wq, wk, wv, wo):
    x16 = np.asarray(x, dtype=np.float16)
    cosT = np.ascontiguousarray(
        np.asarray(cos, np.float32).reshape(S, HD).T)
    sinT = np.ascontiguousarray(
        np.asarray(sin, np.float32).reshape(S, HD).T)
    cs2 = np.ascontiguousarray(np.concatenate([cosT, cosT], axis=0))
    sn2 = np.ascontiguousarray(np.concatenate([sinT, sinT], axis=0))

    def pack_w(w, rows):
        # [128, kt*GC + c] = w.T[kt*128 + p, c] for the row-slice of w
        wT = np.asarray(w, np.float16)[rows, :].T          # [D, GC]
        return np.ascontiguousarray(
            wT.reshape(KT, 128, GC).transpose(1, 0, 2).reshape(128, KT * GC))

    in_maps = []
    for c in range(NCORES):
        b, g = c // GROUPS, c % GROUPS
        rows = slice(g * GC, (g + 1) * GC)
        in_maps.append({
            "xT": np.ascontiguousarray(x16[b].T),
            "wqs": pack_w(wq, rows),
            "wks": pack_w(wk, rows),
            "wvs": pack_w(wv, rows),
            "woT": np.ascontiguousarray(
                np.asarray(wo, np.float16)[:, rows].T),
            "cs2": cs2,
            "sn2": sn2,
        })
    return in_maps


def _run(inputs, trace=False, trace_kwargs=None):
    if "nc" not in _cache:
        _cache["nc"] = _build()
    nc = _cache["nc"]
    in_maps = _shard_inputs(
        inputs["x"], inputs["cos"], inputs["sin"],
        inputs["wq"], inputs["wk"], inputs["wv"], inputs["wo"])
    res = run_bass_kernel_spmd(
        nc, in_maps, list(range(NCORES)), trace=trace,
        **(trace_kwargs or {}))
    full = np.zeros((B, S, D), dtype=np.float32)
    for c in range(NCORES):
        full[c // GROUPS] += res.results[c]["out"].astype(np.float32)
    return full, res


def kernel(**inputs):
    full, _ = _run(inputs, trace=False)
    return full


# revision 13
# speedup vs baseline: 1.1855x; 1.1855x over previous
"""Trainium2 Bass kernel for nn_Attention_84473416778449.

Reference computation (B=2, S=2048, D=1024, H=16, HD=64, fp32):
    q/k/v = x @ w{q,k,v}.T ; RoPE(q, k) ; causal softmax attention ; out @ wo.T

Sharding: 8 cores = (batch 2) x (head-group 4). Each core computes 4 heads of
one batch end-to-end and a partial output projection over its 256 channels;
the host sums the 4 partials per batch.

v2 structure (evidence-driven from the v1 trace):
  - All matmul operands are converted to fp16 on the HOST; DRAM inputs and the
    output are fp16 (cos/sin stay f32). This removes the in-kernel conversion
    copies and the 19us cold-start DMA wall of v1.
  - The PE is warmed with dummy matmuls while the first DMAs land, so the HAM
    clock gate is at 8/8 when real work starts (v1 paid ~15us at 4/8).
  - Score matmuls for the two heads of a qT/kT tile are issued back-to-back to
    disjoint PE row groups (tile_position (0,0)/(64,0)) so they execute
    concurrently, halving score matmul occupancy.
  - The attention loop is software-pipelined: PV(kt-1) is emitted after
    scores(kt), and projection/output-projection work for the next block is
    interleaved into the attention steps as "filler" so the PE never waits on
    the exp (ACT) latency.

Layout (per core):
    xT    [1024, 2048] fp16 (host pre-transposed x[b].T), 512-col slices
    wq/wk/wv staged [128, 8*256] fp16 (host pre-laid-out), woT [256,1024] fp16
    qT/kT [2 x (128, 2048)] two heads per tile (dh on partitions)
    v     [16 x (128, 4*65+64)] s on partitions; per-head 64 cols + ones col
          (the ones column makes the PV matmul emit softmax denominators in
          psum row 64; the PV weight slice is padded to 128 cols)
    scores^T tiles [sk=128, sq<=512] -> exp on ACT -> causal affine_select ->
          PV accumulate -> normalize via broadcast + approx reciprocal.
Diagonal score tiles are trimmed to their unmasked column range; fully
masked tiles are skipped entirely.
"""
import sys

if "/opt/trn_rl_repo" not in sys.path:
    sys.path.insert(0, "/opt/trn_rl_repo")

import numpy as np

import concourse.bass as bass
import concourse.mybir as mybir
import concourse.tile as tile
from concourse import bacc
from concourse.bass_utils import run_bass_kernel_spmd

B, S, D, H, HD = 2, 2048, 1024, 16, 64
NCORES = 8
GROUPS = 4            # head groups
GH = H // GROUPS      # heads per group = 4
GC = GH * HD          # channels per group = 256
KT = D // 128         # 8 k-tiles over D
ST = S // 128         # 16 s-tiles
QB = 4                # sq blocks of 512
QW = S // QB          # 512
VW = GH * (HD + 1)    # 260: v tile payload columns

f32 = mybir.dt.float32
MMDT = mybir.dt.float16   # matmul-operand dtype
Exp = mybir.ActivationFunctionType.Exp
Copy = mybir.ActivationFunctionType.Copy

_cache = {}


def _build():
    nc = bacc.Bacc("TRN2", num_devices=NCORES)

    xT = nc.dram_tensor("xT", [D, S], MMDT, kind="ExternalInput").ap()
    # host-packed: [p, kt*GC + c] = w.T[kt*128 + p, c]
    wqs = nc.dram_tensor("wqs", [128, KT * GC], MMDT, kind="ExternalInput").ap()
    wks = nc.dram_tensor("wks", [128, KT * GC], MMDT, kind="ExternalInput").ap()
    wvs = nc.dram_tensor("wvs", [128, KT * GC], MMDT, kind="ExternalInput").ap()
    woT = nc.dram_tensor("woT", [GC, D], MMDT, kind="ExternalInput").ap()
    cs2 = nc.dram_tensor("cs2", [128, S], MMDT, kind="ExternalInput").ap()
    sn2 = nc.dram_tensor("sn2", [128, S], MMDT, kind="ExternalInput").ap()
    out = nc.dram_tensor("out", [S, D], MMDT, kind="ExternalOutput").ap()

    with tile.TileContext(nc) as tc:
        with tc.tile_pool(name="persist", bufs=1) as pp, \
             tc.tile_pool(name="rope", bufs=3) as rp, \
             tc.tile_pool(name="probs", bufs=6) as wp, \
             tc.tile_pool(name="outsb", bufs=3) as op_, \
             tc.tile_pool(name="small", bufs=3) as sp:

            # ---- PE warm-up: dummy matmuls while the first DMAs land ------
            warm = pp.tile([128, 128], MMDT, tag="warm")
            nc.gpsimd.memset(warm[:], 0.0)
            with tc.tile_pool(name="psW", bufs=1, space="PSUM") as psW:
                wps = psW.tile([128, 128], f32, tag="wps")
                for _ in range(32):
                    nc.tensor.matmul(wps[:], warm[:], warm[:],
                                     start=True, stop=True)

            # ---- first-needed loads go out before anything else ------------
            xTr = [[pp.tile([128, QW], MMDT, tag=f"xT{kt}_{cb}",
                            name=f"xTr{kt}_{cb}") for cb in range(QB)]
                   for kt in range(KT)]

            def load_x_block(cb, eng):
                for kt in range(KT):
                    eng.dma_start(
                        xTr[kt][cb][:],
                        xT[kt * 128:(kt + 1) * 128, cb * QW:(cb + 1) * QW])

            def load_wT(src, eng):
                t = pp.tile([128, KT * GC], MMDT, tag=f"w{src.tensor.name}",
                            name=f"w{src.tensor.name}")
                eng.dma_start(t[:], src)
                return t

            # sync queue carries exactly the first-needed bytes, in order
            wq_s = load_wT(wqs, nc.sync)
            load_x_block(0, nc.sync)
            cs_sb = pp.tile([128, S], MMDT, tag="cs")
            nc.sync.dma_start(cs_sb[:], cs2[:])
            sn_sb = pp.tile([128, S], MMDT, tag="sn")
            nc.sync.dma_start(sn_sb[:], sn2[:])
            load_x_block(1, nc.gpsimd)

            # ---- constants (built in fp32, cast via DVE copy) --------------
            cscratch = pp.tile([128, 128], f32, tag="cscratch")
            nc.gpsimd.memset(cscratch[:], 0.0)
            for blk in range(2):
                sub = cscratch[blk * 64:(blk + 1) * 64,
                               blk * 64:(blk + 1) * 64]
                nc.gpsimd.affine_select(   # -1 where p - f == 32
                    out=sub, in_=sub, pattern=[[-1, 64]], base=-32,
                    channel_multiplier=1,
                    compare_op=mybir.AluOpType.not_equal, fill=-1.0)
                nc.gpsimd.affine_select(   # +1 where f - p == 32
                    out=sub, in_=sub, pattern=[[1, 64]], base=-32,
                    channel_multiplier=-1,
                    compare_op=mybir.AluOpType.not_equal, fill=1.0)
            rt2 = pp.tile([128, 128], MMDT, tag="rt2")
            nc.vector.tensor_copy(rt2[:], cscratch[:])
            ones_f = pp.tile([128, GH], f32, tag="ones_f")
            nc.gpsimd.memset(ones_f[:], 1.0)
            zeros_f = pp.tile([128, 64], f32, tag="zeros_f")
            nc.gpsimd.memset(zeros_f[:], 0.0)

            # ---- remaining loads -------------------------------------------
            wk_s, wv_s = load_wT(wks, nc.gpsimd), load_wT(wvs, nc.gpsimd)
            wo_s = []
            for kt in range(2):
                t = pp.tile([128, D], MMDT, tag=f"wo{kt}", name=f"wo{kt}")
                nc.gpsimd.dma_start(t[:], woT[kt * 128:(kt + 1) * 128, :])
                wo_s.append(t)

            qT = [[pp.tile([128, QW], MMDT, tag=f"qT{i}_{b}",
                           name=f"qT{i}_{b}") for b in range(QB)]
                  for i in range(2)]
            kTt = [[pp.tile([128, QW], MMDT, tag=f"kT{i}_{b}",
                            name=f"kT{i}_{b}") for b in range(QB)]
                   for i in range(2)]
            attnT = [[pp.tile([128, QW], MMDT, tag=f"aT{i}_{b}",
                              name=f"aT{i}_{b}") for b in range(QB)]
                     for i in range(2)]
            v_sb = [pp.tile([128, VW + 64], MMDT, tag=f"v{i}",
                            name=f"v{i}") for i in range(ST)]

            with tc.tile_pool(name="psA", bufs=2, space="PSUM") as psA, \
                 tc.tile_pool(name="psS", bufs=4, space="PSUM") as psS, \
                 tc.tile_pool(name="psO", bufs=2, space="PSUM") as psO:

                def qk_chain_chunks(w_src, dst, hp, sb):
                    cols = slice(sb * QW, (sb + 1) * QW)
                    box = {}

                    def mk_mm(kt):
                        def f():
                            if kt == 0:
                                box['pq'] = psA.tile(
                                    [128, QW], f32, tag="proj",
                                    name=f"pq{hp}_{sb}")
                            nc.tensor.matmul(
                                box['pq'][:],
                                w_src[:, kt * GC + hp * 128:
                                      kt * GC + hp * 128 + 128],
                                xTr[kt][sb][:],
                                start=(kt == 0), stop=(kt == KT - 1))
                        return f

                    def fin():
                        pq = box['pq']
                        tcs = rp.tile([128, QW], MMDT, tag="tcs")
                        nc.vector.tensor_tensor(
                            out=tcs[:], in0=pq[:], in1=cs_sb[:, cols],
                            op=mybir.AluOpType.mult)
                        tsn = rp.tile([128, QW], MMDT, tag="tsn")
                        nc.vector.tensor_tensor(
                            out=tsn[:], in0=pq[:], in1=sn_sb[:, cols],
                            op=mybir.AluOpType.mult)
                        pr = psS.tile([128, QW], f32, tag="score",
                                      name=f"rope{hp}_{sb}")
                        nc.tensor.matmul(pr[:], rt2[:], tsn[:],
                                         start=True, stop=True)
                        nc.vector.tensor_tensor(
                            out=dst[hp][sb][:], in0=pr[:], in1=tcs[:],
                            op=mybir.AluOpType.add)
                    return [mk_mm(kt) for kt in range(KT)] + [fin]

                def v_chain_chunks(st):
                    box = {}

                    def mk_mm(kt):
                        def f():
                            if kt == 0:
                                box['pv'] = psA.tile(
                                    [128, QW], f32, tag="proj",
                                    name=f"pv_{st}")
                            nc.tensor.matmul(
                                box['pv'][:, 0:GC],
                                xTr[kt][st // 4][:, (st % 4) * 128:
                                                 (st % 4) * 128 + 128],
                                wv_s[:, kt * GC:(kt + 1) * GC],
                                start=(kt == 0), stop=(kt == KT - 1))
                        return f

                    def fin():
                        pv = box['pv']
                        vt = v_sb[st]
                        vhe = vt[:, 0:VW].rearrange(
                            "p (h e) -> p h e", e=HD + 1)
                        nc.vector.tensor_copy(vt[:, VW:VW + 64], zeros_f[:])
                        nc.vector.tensor_copy(vhe[:, :, HD:HD + 1], ones_f[:])
                        nc.scalar.activation(
                            vhe[:, :, 0:HD],
                            pv[:, 0:GC].rearrange("p (h d) -> p h d", d=HD),
                            Copy)
                    return [mk_mm(kt) for kt in range(KT)] + [fin]

                def emit_out(st, db):
                    pc = psA.tile([128, QW], f32, tag="proj",
                                  name=f"pc_{st}_{db}")
                    for kt in range(2):
                        nc.tensor.matmul(
                            pc[:],
                            attnT[kt][st // 4][:, (st % 4) * 128:
                                               (st % 4) * 128 + 128],
                            wo_s[kt][:, db * QW:(db + 1) * QW],
                            start=(kt == 0), stop=(kt == 1))
                    ob = op_.tile([128, QW], MMDT, tag="outsb")
                    nc.scalar.activation(ob[:], pc[:], Copy)
                    nc.sync.dma_start(
                        out[st * 128:(st + 1) * 128,
                            db * QW:(db + 1) * QW], ob[:])

                def proj_chunks(sbx):
                    chunks = []
                    for w_src, dst in ((wq_s, qT), (wk_s, kTt)):
                        for hp in range(2):
                            chunks += qk_chain_chunks(w_src, dst, hp, sbx)
                    for st in range(sbx * 4, sbx * 4 + 4):
                        chunks += v_chain_chunks(st)
                    return chunks

                def emit_attention(qb, fillers):
                    nsk = (qb + 1) * 4
                    steps_total = 2 * nsk
                    step = 0
                    fi = 0
                    for hp in range(2):
                        hA, hB = 2 * hp, 2 * hp + 1
                        poA = psO.tile([128, QW], f32, tag="pvacc",
                                       name=f"poA_{hp}_{qb}")
                        poB = psO.tile([128, QW], f32, tag="pvacc",
                                       name=f"poB_{hp}_{qb}")

                        def pv_pair(p):
                            kt, c0, cw, prtA, prtB = p
                            nc.tensor.matmul(
                                poA[:, c0:QW],
                                v_sb[kt][:, hA * (HD + 1):
                                         hA * (HD + 1) + 128],
                                prtA[:, 0:cw],
                                start=(kt == 0), stop=(kt == nsk - 1))
                            nc.tensor.matmul(
                                poB[:, c0:QW],
                                v_sb[kt][:, hB * (HD + 1):
                                         hB * (HD + 1) + 128],
                                prtB[:, 0:cw],
                                start=(kt == 0), stop=(kt == nsk - 1))

                        for kt in range(nsk):
                            c0 = max(0, kt * 128 - qb * QW)
                            cw = QW - c0
                            psa = psS.tile([128, QW], f32, tag="score",
                                           name=f"scA_{hp}_{qb}_{kt}")
                            psb = psS.tile([128, QW], f32, tag="score",
                                           name=f"scB_{hp}_{qb}_{kt}")
                            nc.tensor.matmul(
                                psa[:, 0:cw],
                                kTt[hp][kt // 4][0:64,
                                                 (kt % 4) * 128:
                                                 (kt % 4) * 128 + 128],
                                qT[hp][qb][0:64, c0:QW],
                                start=True, stop=True,
                                tile_position=(0, 0))
                            nc.tensor.matmul(
                                psb[:, 0:cw],
                                kTt[hp][kt // 4][64:128,
                                                 (kt % 4) * 128:
                                                 (kt % 4) * 128 + 128],
                                qT[hp][qb][64:128, c0:QW],
                                start=True, stop=True,
                                tile_position=(64, 0))
                            prtA = wp.tile([128, QW], MMDT, tag="probs",
                                           name=f"prA_{hp}_{qb}_{kt}")
                            prtB = wp.tile([128, QW], MMDT, tag="probs",
                                           name=f"prB_{hp}_{qb}_{kt}")
                            nc.scalar.activation(
                                prtA[:, 0:cw], psa[:, 0:cw], Exp, scale=0.125)
                            nc.scalar.activation(
                                prtB[:, 0:cw], psb[:, 0:cw], Exp, scale=0.125)
                            if kt >= nsk - 4:
                                for prt in (prtA, prtB):
                                    nc.gpsimd.affine_select(
                                        out=prt[:, 0:cw], in_=prt[:, 0:cw],
                                        pattern=[[1, cw]], base=0,
                                        channel_multiplier=-1,
                                        compare_op=mybir.AluOpType.is_ge,
                                        fill=0.0)
                            # one filler matmul covers the exp latency before
                            # this step's PV; the rest follow the PV
                            step += 1
                            want = (len(fillers) * step) // steps_total
                            if fi < want:
                                fillers[fi]()
                                fi += 1
                            pv_pair((kt, c0, cw, prtA, prtB))
                            while fi < want:
                                fillers[fi]()
                                fi += 1
                        for h, po in ((hA, poA), (hB, poB)):
                            d0 = sp.tile([1, QW], f32, tag="den0")
                            nc.vector.tensor_copy(d0[:], po[HD:HD + 1, :])
                            dn = sp.tile([64, QW], f32, tag="denb")
                            nc.gpsimd.partition_broadcast(dn[:], d0[:])
                            rcb = sp.tile([64, QW], f32, tag="recb")
                            nc.vector.reciprocal_approx_fast(
                                out=rcb[:], in_=dn[:])
                            nc.vector.tensor_tensor(
                                out=attnT[hp][qb][(h % 2) * 64:
                                                  (h % 2) * 64 + 64, :],
                                in0=po[0:HD, :], in1=rcb[:],
                                op=mybir.AluOpType.mult)
                    while fi < len(fillers):
                        fillers[fi]()
                        fi += 1

                # sb=0 projections are a dense block (nothing to hide behind)
                for ch in proj_chunks(0):
                    ch()
                for sb in range(QB):
                    if sb + 2 < QB:
                        load_x_block(sb + 2, nc.gpsimd)
                    steps = 2 * (sb + 1) * 4
                    fillers = []
                    if sb > 0:
                        fillers += [
                            (lambda st=st, db=db: emit_out(st, db))
                            for st in range((sb - 1) * 4, sb * 4)
                            for db in range(2)]
                    rest = []
                    if sb + 1 < QB:
                        pch = proj_chunks(sb + 1)
                        # cap interleave rate so the DVE finalize work cannot
                        # back up behind the PE racing through filler matmuls
                        ncap = max(0, 2 * steps - len(fillers))
                        fillers += pch[:ncap]
                        rest = pch[ncap:]
                    emit_attention(sb, fillers)
                    for ch in rest:
                        ch()
                for st in range((QB - 1) * 4, QB * 4):
                    for db in range(2):
                        emit_out(st, db)

    nc.compile()
    return nc


def _shard_inputs(x, cos, sin, wq, wk, wv, wo):
    x16 = np.asarray(x, dtype=np.float16)
    cosT = np.asarray(cos, np.float32).reshape(S, HD).T
    sinT = np.asarray(sin, np.float32).reshape(S, HD).T
    cs2 = np.ascontiguousarray(
        np.concatenate([cosT, cosT], axis=0).astype(np.float16))
    sn2 = np.ascontiguousarray(
        np.concatenate([sinT, sinT], axis=0).astype(np.float16))

    def pack_w(w, rows):
        # [128, kt*GC + c] = w.T[kt*128 + p, c] for the row-slice of w
        wT = np.asarray(w, np.float16)[rows, :].T          # [D, GC]
        return np.ascontiguousarray(
            wT.reshape(KT, 128, GC).transpose(1, 0, 2).reshape(128, KT * GC))

    in_maps = []
    for c in range(NCORES):
        b, g = c // GROUPS, c % GROUPS
        rows = slice(g * GC, (g + 1) * GC)
        in_maps.append({
            "xT": np.ascontiguousarray(x16[b].T),
            "wqs": pack_w(wq, rows),
            "wks": pack_w(wk, rows),
            "wvs": pack_w(wv, rows),
            "woT": np.ascontiguousarray(
                np.asarray(wo, np.float16)[:, rows].T),
            "cs2": cs2,
            "sn2": sn2,
        })
    return in_maps


def _run(inputs, trace=False, trace_kwargs=None):
    if "nc" not in _cache:
        _cache["nc"] = _build()
    nc = _cache["nc"]
    in_maps = _shard_inputs(
        inputs["x"], inputs["cos"], inputs["sin"],
        inputs["wq"], inputs["wk"], inputs["wv"], inputs["wo"])
    res = run_bass_kernel_spmd(
        nc, in_maps, list(range(NCORES)), trace=trace,
        **(trace_kwargs or {}))
    full = np.zeros((B, S, D), dtype=np.float32)
    for c in range(NCORES):
        full[c // GROUPS] += res.results[c]["out"].astype(np.float32)
    return full, res


def kernel(**inputs):
    full, _ = _run(inputs, trace=False)
    return full


# revision 15
# speedup vs baseline: 1.2370x; 1.0435x over previous
"""Trainium2 Bass kernel for nn_Attention_84473416778449.

Reference computation (B=2, S=2048, D=1024, H=16, HD=64, fp32):
    q/k/v = x @ w{q,k,v}.T ; RoPE(q, k) ; causal softmax attention ; out @ wo.T

Sharding: 8 cores = (batch 2) x (head-group 4). Each core computes 4 heads of
one batch end-to-end and a partial output projection over its 256 channels;
the host sums the 4 partials per batch.

v4 structure (evidence-driven through three traced iterations):
  - fp16 everywhere on the host side: x is pre-packed per 512-row block so one
    block loads with a SINGLE dma descriptor instruction (descriptor issue on
    the queueing engine costs ~650ns each; v3 spent 20us+ issuing 128KB DMAs).
  - DMA queues balanced: sync carries wq/x0/cos/sin/x1 (first-needed, in
    order), gpsimd carries wk/wv/wo/x2/x3.
  - PE warm-up matmuls bridge the framework preamble to the first real matmul
    so the HAM clock gate reaches 8/8 and stays there.
  - Score matmuls for the two heads of a qT/kT tile go to disjoint PE row
    groups (tile_position (0,0)/(64,0)) and execute concurrently (measured
    4ns apart).
  - All deferred work is emitted as fine-grained chunks interleaved into the
    attention steps; the RoPE matmul of each projection chain is emitted a
    few chunks later than its DVE inputs so it never head-of-line blocks the
    PE queue.
"""
import sys

if "/opt/trn_rl_repo" not in sys.path:
    sys.path.insert(0, "/opt/trn_rl_repo")

import numpy as np

import concourse.bass as bass
import concourse.mybir as mybir
import concourse.tile as tile
from concourse import bacc
from concourse.bass_utils import run_bass_kernel_spmd

B, S, D, H, HD = 2, 2048, 1024, 16, 64
NCORES = 8
GROUPS = 4            # head groups
GH = H // GROUPS      # heads per group = 4
GC = GH * HD          # channels per group = 256
KT = D // 128         # 8 k-tiles over D
ST = S // 128         # 16 s-tiles
QB = 4                # sq blocks of 512
QW = S // QB          # 512
XW = KT * QW          # 4096: packed x block width
VW = GH * (HD + 1)    # 260: v tile payload columns

f32 = mybir.dt.float32
MMDT = mybir.dt.float16   # matmul-operand dtype
Exp = mybir.ActivationFunctionType.Exp
Copy = mybir.ActivationFunctionType.Copy

_cache = {}


def _build():
    nc = bacc.Bacc("TRN2", num_devices=NCORES)

    # host-packed: row-block cb holds [p, kt*QW + j] = x[b].T[kt*128+p, cb*QW+j]
    xB = nc.dram_tensor("xB", [QB * 128, XW], MMDT, kind="ExternalInput").ap()
    # host-packed: [p, kt*GC + c] = w.T[kt*128 + p, c]
    wqs = nc.dram_tensor("wqs", [128, KT * GC], MMDT, kind="ExternalInput").ap()
    wks = nc.dram_tensor("wks", [128, KT * GC], MMDT, kind="ExternalInput").ap()
    wvs = nc.dram_tensor("wvs", [128, KT * GC], MMDT, kind="ExternalInput").ap()
    woT = nc.dram_tensor("woT", [GC, D], MMDT, kind="ExternalInput").ap()
    cs2 = nc.dram_tensor("cs2", [128, S], MMDT, kind="ExternalInput").ap()
    sn2 = nc.dram_tensor("sn2", [128, S], MMDT, kind="ExternalInput").ap()
    out = nc.dram_tensor("out", [S, D], MMDT, kind="ExternalOutput").ap()

    with tile.TileContext(nc) as tc:
        with tc.tile_pool(name="persist", bufs=1) as pp, \
             tc.tile_pool(name="rope", bufs=3) as rp, \
             tc.tile_pool(name="probs", bufs=6) as wp, \
             tc.tile_pool(name="outsb", bufs=2) as op_, \
             tc.tile_pool(name="small", bufs=3) as sp:

            # ---- PE warm-up: dummy matmuls while the first DMAs land ------
            warm = pp.tile([128, 128], MMDT, tag="warm")
            nc.gpsimd.memset(warm[:], 0.0)
            with tc.tile_pool(name="psW", bufs=1, space="PSUM") as psW:
                wps = psW.tile([128, 128], f32, tag="wps")
                for _ in range(22):
                    nc.tensor.matmul(wps[:], warm[:], warm[:],
                                     start=True, stop=True)

            # ---- loads: sync carries the first-needed bytes in order -------
            def load_w(src, eng):
                t = pp.tile([128, KT * GC], MMDT, tag=f"w{src.tensor.name}",
                            name=f"w{src.tensor.name}")
                eng.dma_start(t[:], src)
                return t

            xb = [pp.tile([128, XW], MMDT, tag=f"xb{cb}", name=f"xb{cb}")
                  for cb in range(QB)]

            def xs(kt, cb):
                return xb[cb][:, kt * QW:(kt + 1) * QW]

            wq_s = load_w(wqs, nc.sync)
            nc.sync.dma_start(xb[0][:], xB[0:128, :])
            cs_sb = pp.tile([128, S], MMDT, tag="cs")
            nc.sync.dma_start(cs_sb[:], cs2[:])
            sn_sb = pp.tile([128, S], MMDT, tag="sn")
            nc.sync.dma_start(sn_sb[:], sn2[:])
            nc.sync.dma_start(xb[1][:], xB[128:256, :])

            # ---- constants (built in fp32, cast via DVE copy) --------------
            cscratch = pp.tile([128, 128], f32, tag="cscratch")
            nc.gpsimd.memset(cscratch[:], 0.0)
            for blk in range(2):
                sub = cscratch[blk * 64:(blk + 1) * 64,
                               blk * 64:(blk + 1) * 64]
                nc.gpsimd.affine_select(   # -1 where p - f == 32
                    out=sub, in_=sub, pattern=[[-1, 64]], base=-32,
                    channel_multiplier=1,
                    compare_op=mybir.AluOpType.not_equal, fill=-1.0)
                nc.gpsimd.affine_select(   # +1 where f - p == 32
                    out=sub, in_=sub, pattern=[[1, 64]], base=-32,
                    channel_multiplier=-1,
                    compare_op=mybir.AluOpType.not_equal, fill=1.0)
            rt2 = pp.tile([128, 128], MMDT, tag="rt2")
            nc.vector.tensor_copy(rt2[:], cscratch[:])
            ones_f = pp.tile([128, GH], f32, tag="ones_f")
            nc.gpsimd.memset(ones_f[:], 1.0)
            zeros_f = pp.tile([128, 64], f32, tag="zeros_f")
            nc.gpsimd.memset(zeros_f[:], 0.0)

            # ---- remaining loads (gpsimd queue) ----------------------------
            wk_s, wv_s = load_w(wks, nc.gpsimd), load_w(wvs, nc.gpsimd)
            wo_s = []
            for kt in range(2):
                t = pp.tile([128, D], MMDT, tag=f"wo{kt}", name=f"wo{kt}")
                nc.gpsimd.dma_start(t[:], woT[kt * 128:(kt + 1) * 128, :])
                wo_s.append(t)
            nc.gpsimd.dma_start(xb[2][:], xB[256:384, :])
            nc.gpsimd.dma_start(xb[3][:], xB[384:512, :])

            qT = [[pp.tile([128, QW], MMDT, tag=f"qT{i}_{b}",
                           name=f"qT{i}_{b}") for b in range(QB)]
                  for i in range(2)]
            kTt = [[pp.tile([128, QW], MMDT, tag=f"kT{i}_{b}",
                            name=f"kT{i}_{b}") for b in range(QB)]
                   for i in range(2)]
            attnT = [[pp.tile([128, QW], MMDT, tag=f"aT{i}_{b}",
                              name=f"aT{i}_{b}") for b in range(QB)]
                     for i in range(2)]
            v_sb = [pp.tile([128, VW + 64], MMDT, tag=f"v{i}",
                            name=f"v{i}") for i in range(ST)]

            with tc.tile_pool(name="psA", bufs=2, space="PSUM") as psA, \
                 tc.tile_pool(name="psS", bufs=4, space="PSUM") as psS, \
                 tc.tile_pool(name="psO", bufs=2, space="PSUM") as psO:

                def qk_chain_chunks(w_src, dst, hp, sb):
                    """[mm x8, fin_dve] plus a separate fin_rope chunk that the
                    caller emits a few chunks later (it contains the PE rope
                    matmul, which must not wait at the head of the PE queue)."""
                    cols = slice(sb * QW, (sb + 1) * QW)
                    box = {}

                    def mk_mm(kt):
                        def f():
                            if kt == 0:
                                box['pq'] = psA.tile(
                                    [128, QW], f32, tag="proj",
                                    name=f"pq{hp}_{sb}")
                            nc.tensor.matmul(
                                box['pq'][:],
                                w_src[:, kt * GC + hp * 128:
                                      kt * GC + hp * 128 + 128],
                                xs(kt, sb),
                                start=(kt == 0), stop=(kt == KT - 1))
                        return f

                    def fin_dve():
                        pq = box['pq']
                        box['tcs'] = rp.tile([128, QW], MMDT, tag="tcs",
                                              name=f"tcs{hp}_{sb}")
                        nc.vector.tensor_tensor(
                            out=box['tcs'][:], in0=pq[:], in1=cs_sb[:, cols],
                            op=mybir.AluOpType.mult)
                        box['tsn'] = rp.tile([128, QW], MMDT, tag="tsn",
                                              name=f"tsn{hp}_{sb}")
                        nc.vector.tensor_tensor(
                            out=box['tsn'][:], in0=pq[:], in1=sn_sb[:, cols],
                            op=mybir.AluOpType.mult)

                    def fin_rope():
                        pr = psS.tile([128, QW], f32, tag="score",
                                      name=f"rope{hp}_{sb}")
                        nc.tensor.matmul(pr[:], rt2[:], box['tsn'][:],
                                         start=True, stop=True)
                        nc.vector.tensor_tensor(
                            out=dst[hp][sb][:], in0=pr[:], in1=box['tcs'][:],
                            op=mybir.AluOpType.add)
                    return [mk_mm(kt) for kt in range(KT)] + [fin_dve], fin_rope

                def v_chain_chunks(st):
                    box = {}

                    def mk_mm(kt):
                        def f():
                            if kt == 0:
                                box['pv'] = psA.tile(
                                    [128, QW], f32, tag="proj",
                                    name=f"pv_{st}")
                            nc.tensor.matmul(
                                box['pv'][:, 0:GC],
                                xs(kt, st // 4)[:, (st % 4) * 128:
                                                (st % 4) * 128 + 128],
                                wv_s[:, kt * GC:(kt + 1) * GC],
                                start=(kt == 0), stop=(kt == KT - 1))
                        return f

                    def fin():
                        pv = box['pv']
                        vt = v_sb[st]
                        vhe = vt[:, 0:VW].rearrange(
                            "p (h e) -> p h e", e=HD + 1)
                        nc.vector.tensor_copy(vt[:, VW:VW + 64], zeros_f[:])
                        nc.vector.tensor_copy(vhe[:, :, HD:HD + 1], ones_f[:])
                        nc.scalar.activation(
                            vhe[:, :, 0:HD],
                            pv[:, 0:GC].rearrange("p (h d) -> p h d", d=HD),
                            Copy)
                    return [mk_mm(kt) for kt in range(KT)] + [fin]

                def proj_chunks(sbx):
                    """Chunk list for block sbx with each chain's rope matmul
                    deferred into the following chain's matmul stream."""
                    chunks = []
                    pending = []

                    def flush():
                        while pending:
                            chunks.append(pending.pop(0))

                    for w_src, dst in ((wq_s, qT), (wk_s, kTt)):
                        for hp in range(2):
                            main, fin_rope = qk_chain_chunks(
                                w_src, dst, hp, sbx)
                            chunks += main[:3]
                            flush()
                            chunks += main[3:]
                            pending.append(fin_rope)
                    for st in range(sbx * 4, sbx * 4 + 4):
                        vc = v_chain_chunks(st)
                        chunks += vc[:3]
                        flush()
                        chunks += vc[3:]
                    return chunks

                def out_chunks(st):
                    box = {}

                    def mms(db):
                        def f():
                            box[db] = psA.tile([128, QW], f32, tag="proj",
                                               name=f"pc_{st}_{db}")
                            if db == 0:
                                box['ob'] = op_.tile([128, 2 * QW], MMDT,
                                                     tag="outsb",
                                                     name=f"ob_{st}")
                            for kt in range(2):
                                nc.tensor.matmul(
                                    box[db][:],
                                    attnT[kt][st // 4][:, (st % 4) * 128:
                                                       (st % 4) * 128 + 128],
                                    wo_s[kt][:, db * QW:(db + 1) * QW],
                                    start=(kt == 0), stop=(kt == 1))
                        return f

                    def cp(db):
                        def f():
                            nc.scalar.activation(
                                box['ob'][:, db * QW:(db + 1) * QW],
                                box[db][:], Copy)
                            if db == 1:
                                nc.sync.dma_start(
                                    out[st * 128:(st + 1) * 128, :],
                                    box['ob'][:])
                        return f
                    return [mms(0), cp(0), mms(1), cp(1)]

                def merge_prop(a, b):
                    res, ia, ib = [], 0, 0
                    la, lb = max(len(a), 1), max(len(b), 1)
                    while ia < len(a) or ib < len(b):
                        if ib >= len(b) or (ia < len(a) and
                                            ia * lb <= ib * la):
                            res.append(a[ia]); ia += 1
                        else:
                            res.append(b[ib]); ib += 1
                    return res

                def emit_attention(qb, fillers):
                    nsk = (qb + 1) * 4
                    steps_total = 2 * nsk
                    step = 0
                    fi = 0
                    for hp in range(2):
                        hA, hB = 2 * hp, 2 * hp + 1
                        poA = psO.tile([128, QW], f32, tag="pvacc",
                                       name=f"poA_{hp}_{qb}")
                        poB = psO.tile([128, QW], f32, tag="pvacc",
                                       name=f"poB_{hp}_{qb}")

                        def pv_pair(kt, c0, cw, prtA, prtB):
                            nc.tensor.matmul(
                                poA[:, c0:QW],
                                v_sb[kt][:, hA * (HD + 1):
                                         hA * (HD + 1) + 128],
                                prtA[:, 0:cw],
                                start=(kt == 0), stop=(kt == nsk - 1))
                            nc.tensor.matmul(
                                poB[:, c0:QW],
                                v_sb[kt][:, hB * (HD + 1):
                                         hB * (HD + 1) + 128],
                                prtB[:, 0:cw],
                                start=(kt == 0), stop=(kt == nsk - 1))

                        for kt in range(nsk):
                            c0 = max(0, kt * 128 - qb * QW)
                            cw = QW - c0
                            psa = psS.tile([128, QW], f32, tag="score",
                                           name=f"scA_{hp}_{qb}_{kt}")
                            psb = psS.tile([128, QW], f32, tag="score",
                                           name=f"scB_{hp}_{qb}_{kt}")
                            nc.tensor.matmul(
                                psa[:, 0:cw],
                                kTt[hp][kt // 4][0:64,
                                                 (kt % 4) * 128:
                                                 (kt % 4) * 128 + 128],
                                qT[hp][qb][0:64, c0:QW],
                                start=True, stop=True,
                                tile_position=(0, 0))
                            nc.tensor.matmul(
                                psb[:, 0:cw],
                                kTt[hp][kt // 4][64:128,
                                                 (kt % 4) * 128:
                                                 (kt % 4) * 128 + 128],
                                qT[hp][qb][64:128, c0:QW],
                                start=True, stop=True,
                                tile_position=(64, 0))
                            prtA = wp.tile([128, QW], MMDT, tag="probs",
                                           name=f"prA_{hp}_{qb}_{kt}")
                            prtB = wp.tile([128, QW], MMDT, tag="probs",
                                           name=f"prB_{hp}_{qb}_{kt}")
                            nc.scalar.activation(
                                prtA[:, 0:cw], psa[:, 0:cw], Exp, scale=0.125)
                            nc.scalar.activation(
                                prtB[:, 0:cw], psb[:, 0:cw], Exp, scale=0.125)
                            if kt >= nsk - 4:
                                for prt in (prtA, prtB):
                                    nc.gpsimd.affine_select(
                                        out=prt[:, 0:cw], in_=prt[:, 0:cw],
                                        pattern=[[1, cw]], base=0,
                                        channel_multiplier=-1,
                                        compare_op=mybir.AluOpType.is_ge,
                                        fill=0.0)
                            # one filler matmul covers the exp latency before
                            # this step's PV; the rest follow the PV
                            step += 1
                            want = (len(fillers) * step) // steps_total
                            if fi < want:
                                fillers[fi]()
                                fi += 1
                            pv_pair(kt, c0, cw, prtA, prtB)
                            while fi < want:
                                fillers[fi]()
                                fi += 1
                        for h, po in ((hA, poA), (hB, poB)):
                            d0 = sp.tile([1, QW], f32, tag="den0")
                            nc.vector.tensor_copy(d0[:], po[HD:HD + 1, :])
                            dn = sp.tile([64, QW], f32, tag="denb")
                            nc.gpsimd.partition_broadcast(dn[:], d0[:])
                            rcb = sp.tile([64, QW], f32, tag="recb")
                            nc.vector.reciprocal_approx_fast(
                                out=rcb[:], in_=dn[:])
                            nc.vector.tensor_tensor(
                                out=attnT[hp][qb][(h % 2) * 64:
                                                  (h % 2) * 64 + 64, :],
                                in0=po[0:HD, :], in1=rcb[:],
                                op=mybir.AluOpType.mult)
                    while fi < len(fillers):
                        fillers[fi]()
                        fi += 1

                # sb=0 projections are a dense block (nothing to hide behind)
                for ch in proj_chunks(0):
                    ch()
                for sb in range(QB):
                    steps = 2 * (sb + 1) * 4
                    outs = []
                    if sb > 0:
                        for st in range((sb - 1) * 4, sb * 4):
                            outs += out_chunks(st)
                    rest = []
                    projs = []
                    if sb + 1 < QB:
                        pch = proj_chunks(sb + 1)
                        # cap interleave rate so deferred DVE work cannot
                        # back up behind the PE racing through fillers
                        ncap = max(0, 2 * steps - len(outs))
                        projs = pch[:ncap]
                        rest = pch[ncap:]
                    emit_attention(sb, merge_prop(projs, outs))
                    for ch in rest:
                        ch()
                for st in range((QB - 1) * 4, QB * 4):
                    for ch in out_chunks(st):
                        ch()

    nc.compile()
    return nc


def _shard_inputs(x, cos, sin, wq, wk, wv, wo):
    x16 = np.asarray(x, dtype=np.float16)
    cosT = np.asarray(cos, np.float32).reshape(S, HD).T
    sinT = np.asarray(sin, np.float32).reshape(S, HD).T
    cs2 = np.ascontiguousarray(
        np.concatenate([cosT, cosT], axis=0).astype(np.float16))
    sn2 = np.ascontiguousarray(
        np.concatenate([sinT, sinT], axis=0).astype(np.float16))

    def pack_w(w, rows):
        # [128, kt*GC + c] = w.T[kt*128 + p, c] for the row-slice of w
        wT = np.asarray(w, np.float16)[rows, :].T          # [D, GC]
        return np.ascontiguousarray(
            wT.reshape(KT, 128, GC).transpose(1, 0, 2).reshape(128, KT * GC))

    def pack_x(xb):
        # [cb*128 + p, kt*QW + j] = x.T[kt*128 + p, cb*QW + j]
        xT = xb.T                                          # [D, S]
        return np.ascontiguousarray(
            xT.reshape(KT, 128, QB, QW).transpose(2, 1, 0, 3)
              .reshape(QB * 128, KT * QW))

    in_maps = []
    for c in range(NCORES):
        b, g = c // GROUPS, c % GROUPS
        rows = slice(g * GC, (g + 1) * GC)
        in_maps.append({
            "xB": pack_x(x16[b]),
            "wqs": pack_w(wq, rows),
            "wks": pack_w(wk, rows),
            "wvs": pack_w(wv, rows),
            "woT": np.ascontiguousarray(
                np.asarray(wo, np.float16)[:, rows].T),
            "cs2": cs2,
            "sn2": sn2,
        })
    return in_maps


def _run(inputs, trace=False, trace_kwargs=None):
    if "nc" not in _cache:
        _cache["nc"] = _build()
    nc = _cache["nc"]
    in_maps = _shard_inputs(
        inputs["x"], inputs["cos"], inputs["sin"],
        inputs["wq"], inputs["wk"], inputs["wv"], inputs["wo"])
    res = run_bass_kernel_spmd(
        nc, in_maps, list(range(NCORES)), trace=trace,
        **(trace_kwargs or {}))
    full = np.zeros((B, S, D), dtype=np.float32)
    for c in range(NCORES):
        full[c // GROUPS] += res.results[c]["out"].astype(np.float32)
    return full, res


def kernel(**inputs):
    full, _ = _run(inputs, trace=False)
    return full


# revision 17
# speedup vs baseline: 1.3564x; 1.0965x over previous
"""Trainium2 Bass kernel for nn_Attention_84473416778449.

Reference computation (B=2, S=2048, D=1024, H=16, HD=64, fp32):
    q/k/v = x @ w{q,k,v}.T ; RoPE(q, k) ; causal softmax attention ; out @ wo.T

Sharding: 8 cores = (batch 2) x (head-group 4). Each core computes 4 heads of
one batch end-to-end and a partial output projection over its 256 channels;
the host sums the 4 partials per batch.

v5 structure (evidence-driven through four traced iterations):
  - fp16 host-packed inputs; x blocks load as single dma descriptors.
  - Loads spread over the three DMA-capable queues (sync/scalar/gpsimd),
    ordered by first-need so the PE never bubbles after the ramp
    (per-queue dma bandwidth measured ~115GB/s).
  - PE warm-up matmuls bridge the framework preamble so the HAM clock gate
    reaches 8/8 before real work.
  - Score matmuls for a head pair go to disjoint PE row groups
    (tile_position (0,0)/(64,0)) and run concurrently (measured 4ns apart);
    both write halves of one 2-bank psum tile so ONE batched exp covers the
    pair (halves the ACT per-instruction overhead).
  - V tiles carry 64 replicated ones-columns per head, so the PV matmul
    emits the softmax denominator replicated across psum rows 64:128 and
    normalization is just reciprocal+multiply on the DVE (no psum row copy,
    no gpsimd partition broadcast on the critical path).
  - The RoPE matmul reuses its own chain's psA tile (dead after the cos/sin
    multiplies) and is emitted a few chunks late so it never head-of-line
    blocks the PE queue.
"""
import sys

if "/opt/trn_rl_repo" not in sys.path:
    sys.path.insert(0, "/opt/trn_rl_repo")

import numpy as np

import concourse.bass as bass
import concourse.mybir as mybir
import concourse.tile as tile
from concourse import bacc
from concourse.bass_utils import run_bass_kernel_spmd

B, S, D, H, HD = 2, 2048, 1024, 16, 64
NCORES = 8
GROUPS = 4            # head groups
GH = H // GROUPS      # heads per group = 4
GC = GH * HD          # channels per group = 256
KT = D // 128         # 8 k-tiles over D
ST = S // 128         # 16 s-tiles
QB = 4                # sq blocks of 512
QW = S // QB          # 512
XW = KT * QW          # 4096: packed x block width

f32 = mybir.dt.float32
MMDT = mybir.dt.float16   # matmul-operand dtype
Exp = mybir.ActivationFunctionType.Exp
Copy = mybir.ActivationFunctionType.Copy

_cache = {}


def _build():
    nc = bacc.Bacc("TRN2", num_devices=NCORES)

    # host-packed: row-block cb holds [p, kt*QW + j] = x[b].T[kt*128+p, cb*QW+j]
    xB = nc.dram_tensor("xB", [QB * 128, XW], MMDT, kind="ExternalInput").ap()
    # host-packed: [p, kt*GC + c] = w.T[kt*128 + p, c]
    wqs = nc.dram_tensor("wqs", [128, KT * GC], MMDT, kind="ExternalInput").ap()
    wks = nc.dram_tensor("wks", [128, KT * GC], MMDT, kind="ExternalInput").ap()
    wvs = nc.dram_tensor("wvs", [128, KT * GC], MMDT, kind="ExternalInput").ap()
    woT = nc.dram_tensor("woT", [GC, D], MMDT, kind="ExternalInput").ap()
    cs2 = nc.dram_tensor("cs2", [128, S], MMDT, kind="ExternalInput").ap()
    sn2 = nc.dram_tensor("sn2", [128, S], MMDT, kind="ExternalInput").ap()
    out = nc.dram_tensor("out", [S, D], MMDT, kind="ExternalOutput").ap()

    with tile.TileContext(nc) as tc:
        with tc.tile_pool(name="persist", bufs=1) as pp, \
             tc.tile_pool(name="rope", bufs=3) as rp, \
             tc.tile_pool(name="probs", bufs=6) as wp, \
             tc.tile_pool(name="outsb", bufs=2) as op_, \
             tc.tile_pool(name="small", bufs=3) as sp:

            xb = [pp.tile([128, XW], MMDT, tag=f"xb{cb}", name=f"xb{cb}")
                  for cb in range(QB)]

            def xs(kt, cb):
                return xb[cb][:, kt * QW:(kt + 1) * QW]

            def load_w(src, eng):
                t = pp.tile([128, KT * GC], MMDT, tag=f"w{src.tensor.name}",
                            name=f"w{src.tensor.name}")
                eng.dma_start(t[:], src)
                return t

            # ---- loads spread across the three DMA queues by first-need ----
            warm = pp.tile([128, 128], MMDT, tag="warm")
            nc.gpsimd.memset(warm[:], 0.0)
            nc.gpsimd.dma_start(xb[0][:, 0:XW // 2], xB[0:128, 0:XW // 2])
            wq_s = load_w(wqs, nc.sync)
            nc.scalar.dma_start(xb[0][:, XW // 2:XW], xB[0:128, XW // 2:XW])
            cs_sb = pp.tile([128, S], MMDT, tag="cs")
            nc.sync.dma_start(cs_sb[:], cs2[:])
            sn_sb = pp.tile([128, S], MMDT, tag="sn")
            nc.scalar.dma_start(sn_sb[:], sn2[:])
            wk_s = load_w(wks, nc.gpsimd)
            wv_s = load_w(wvs, nc.scalar)
            wo_s = []
            for kt in range(2):
                t = pp.tile([128, D], MMDT, tag=f"wo{kt}", name=f"wo{kt}")
                nc.sync.dma_start(t[:], woT[kt * 128:(kt + 1) * 128, :])
                wo_s.append(t)
            nc.gpsimd.dma_start(xb[1][:], xB[128:256, :])
            nc.gpsimd.dma_start(xb[2][:], xB[256:384, :])
            nc.gpsimd.dma_start(xb[3][:], xB[384:512, :])

            # ---- PE warm-up: dummy matmuls while the first DMAs land ------
            with tc.tile_pool(name="psW", bufs=1, space="PSUM") as psW:
                wps = psW.tile([128, 128], f32, tag="wps")
                for _ in range(34):
                    nc.tensor.matmul(wps[:], warm[:], warm[:],
                                     start=True, stop=True)

            # ---- constants -------------------------------------------------
            cscratch = pp.tile([128, 128], f32, tag="cscratch")
            nc.gpsimd.memset(cscratch[:], 0.0)
            for blk in range(2):
                sub = cscratch[blk * 64:(blk + 1) * 64,
                               blk * 64:(blk + 1) * 64]
                nc.gpsimd.affine_select(   # -1 where p - f == 32
                    out=sub, in_=sub, pattern=[[-1, 64]], base=-32,
                    channel_multiplier=1,
                    compare_op=mybir.AluOpType.not_equal, fill=-1.0)
                nc.gpsimd.affine_select(   # +1 where f - p == 32
                    out=sub, in_=sub, pattern=[[1, 64]], base=-32,
                    channel_multiplier=-1,
                    compare_op=mybir.AluOpType.not_equal, fill=1.0)
            rt2 = pp.tile([128, 128], MMDT, tag="rt2")
            nc.vector.tensor_copy(rt2[:], cscratch[:])

            qT = [[pp.tile([128, QW], MMDT, tag=f"qT{i}_{b}",
                           name=f"qT{i}_{b}") for b in range(QB)]
                  for i in range(2)]
            kTt = [[pp.tile([128, QW], MMDT, tag=f"kT{i}_{b}",
                            name=f"kT{i}_{b}") for b in range(QB)]
                   for i in range(2)]
            attnT = [[pp.tile([128, QW], MMDT, tag=f"aT{i}_{b}",
                              name=f"aT{i}_{b}") for b in range(QB)]
                     for i in range(2)]
            # per head h: cols h*128+0:64 = v payload, h*128+64:128 = ones
            # (PV then emits the denominator replicated on psum rows 64:128)
            v_sb = [pp.tile([128, 4 * 128], MMDT, tag=f"v{i}",
                            name=f"v{i}") for i in range(ST)]
            ones256 = pp.tile([128, 256], f32, tag="ones256")
            nc.gpsimd.memset(ones256[:], 1.0)
            for vt in v_sb:
                nc.vector.tensor_copy(
                    vt[:].rearrange("p (h e) -> p h e", e=128)[:, :, HD:128],
                    ones256[:].rearrange("p (h e) -> p h e", e=HD))

            with tc.tile_pool(name="psA", bufs=2, space="PSUM") as psA, \
                 tc.tile_pool(name="psS", bufs=2, space="PSUM") as psS, \
                 tc.tile_pool(name="psO", bufs=2, space="PSUM") as psO:

                def qk_chain_chunks(w_src, dst, hp, sb):
                    """[mm x8, fin_dve] plus a separate fin_rope chunk that the
                    caller emits a few chunks later (it holds the PE rope
                    matmul, which must not wait at the head of the PE queue).
                    The rope matmul reuses the chain's own psA tile."""
                    cols = slice(sb * QW, (sb + 1) * QW)
                    box = {}

                    def mk_mm(kt):
                        def f():
                            if kt == 0:
                                box['pq'] = psA.tile(
                                    [128, QW], f32, tag="proj",
                                    name=f"pq{hp}_{sb}")
                            nc.tensor.matmul(
                                box['pq'][:],
                                w_src[:, kt * GC + hp * 128:
                                      kt * GC + hp * 128 + 128],
                                xs(kt, sb),
                                start=(kt == 0), stop=(kt == KT - 1))
                        return f

                    def fin_dve():
                        pq = box['pq']
                        box['tcs'] = rp.tile([128, QW], MMDT, tag="tcs",
                                             name=f"tcs{hp}_{sb}")
                        nc.vector.tensor_tensor(
                            out=box['tcs'][:], in0=pq[:], in1=cs_sb[:, cols],
                            op=mybir.AluOpType.mult)
                        box['tsn'] = rp.tile([128, QW], MMDT, tag="tsn",
                                             name=f"tsn{hp}_{sb}")
                        nc.vector.tensor_tensor(
                            out=box['tsn'][:], in0=pq[:], in1=sn_sb[:, cols],
                            op=mybir.AluOpType.mult)

                    def fin_rope():
                        pq = box['pq']
                        nc.tensor.matmul(pq[:], rt2[:], box['tsn'][:],
                                         start=True, stop=True)
                        nc.vector.tensor_tensor(
                            out=dst[hp][sb][:], in0=pq[:], in1=box['tcs'][:],
                            op=mybir.AluOpType.add)
                    return [mk_mm(kt) for kt in range(KT)] + [fin_dve], fin_rope

                def v_chain_chunks(st):
                    box = {}

                    def mk_mm(kt):
                        def f():
                            if kt == 0:
                                box['pv'] = psA.tile(
                                    [128, QW], f32, tag="proj",
                                    name=f"pv_{st}")
                            nc.tensor.matmul(
                                box['pv'][:, 0:GC],
                                xs(kt, st // 4)[:, (st % 4) * 128:
                                                (st % 4) * 128 + 128],
                                wv_s[:, kt * GC:(kt + 1) * GC],
                                start=(kt == 0), stop=(kt == KT - 1))
                        return f

                    def fin():
                        vhe = v_sb[st][:].rearrange("p (h e) -> p h e", e=128)
                        nc.scalar.activation(
                            vhe[:, :, 0:HD],
                            box['pv'][:, 0:GC].rearrange(
                                "p (h d) -> p h d", d=HD),
                            Copy)
                    return [mk_mm(kt) for kt in range(KT)] + [fin]

                def proj_chunks(sbx):
                    """Chunk list for block sbx with each chain's rope matmul
                    deferred into the following chain's matmul stream."""
                    chunks = []
                    pending = []

                    def flush():
                        while pending:
                            chunks.append(pending.pop(0))

                    for w_src, dst in ((wq_s, qT), (wk_s, kTt)):
                        for hp in range(2):
                            main, fin_rope = qk_chain_chunks(
                                w_src, dst, hp, sbx)
                            chunks += main[:3]
                            flush()
                            chunks += main[3:]
                            pending.append(fin_rope)
                    for st in range(sbx * 4, sbx * 4 + 4):
                        vc = v_chain_chunks(st)
                        chunks += vc[:3]
                        flush()
                        chunks += vc[3:]
                    return chunks

                def out_chunks(st):
                    box = {}

                    def mms(db):
                        def f():
                            box[db] = psA.tile([128, QW], f32, tag="proj",
                                               name=f"pc_{st}_{db}")
                            if db == 0:
                                box['ob'] = op_.tile([128, 2 * QW], MMDT,
                                                     tag="outsb",
                                                     name=f"ob_{st}")
                            for kt in range(2):
                                nc.tensor.matmul(
                                    box[db][:],
                                    attnT[kt][st // 4][:, (st % 4) * 128:
                                                       (st % 4) * 128 + 128],
                                    wo_s[kt][:, db * QW:(db + 1) * QW],
                                    start=(kt == 0), stop=(kt == 1))
                        return f

                    def cp(db):
                        def f():
                            nc.scalar.activation(
                                box['ob'][:, db * QW:(db + 1) * QW],
                                box[db][:], Copy)
                            if db == 1:
                                nc.sync.dma_start(
                                    out[st * 128:(st + 1) * 128, :],
                                    box['ob'][:])
                        return f
                    return [mms(0), cp(0), mms(1), cp(1)]

                def merge_prop(a, b):
                    res, ia, ib = [], 0, 0
                    la, lb = max(len(a), 1), max(len(b), 1)
                    while ia < len(a) or ib < len(b):
                        if ib >= len(b) or (ia < len(a) and
                                            ia * lb <= ib * la):
                            res.append(a[ia]); ia += 1
                        else:
                            res.append(b[ib]); ib += 1
                    return res

                def emit_attention(qb, fillers):
                    nsk = (qb + 1) * 4
                    steps_total = 2 * nsk
                    step = 0
                    fi = 0
                    for hp in range(2):
                        hA, hB = 2 * hp, 2 * hp + 1
                        poA = psO.tile([128, QW], f32, tag="pvacc",
                                       name=f"poA_{hp}_{qb}")
                        poB = psO.tile([128, QW], f32, tag="pvacc",
                                       name=f"poB_{hp}_{qb}")

                        def pv_pair(kt, c0, cw, prt):
                            nc.tensor.matmul(
                                poA[:, c0:QW],
                                v_sb[kt][:, hA * 128:hA * 128 + 128],
                                prt[:, 0:cw],
                                start=(kt == 0), stop=(kt == nsk - 1))
                            nc.tensor.matmul(
                                poB[:, c0:QW],
                                v_sb[kt][:, hB * 128:hB * 128 + 128],
                                prt[:, QW:QW + cw],
                                start=(kt == 0), stop=(kt == nsk - 1))

                        for kt in range(nsk):
                            c0 = max(0, kt * 128 - qb * QW)
                            cw = QW - c0
                            ps2 = psS.tile([128, 2 * QW], f32, tag="score",
                                           name=f"sc_{hp}_{qb}_{kt}")
                            nc.tensor.matmul(
                                ps2[:, 0:cw],
                                kTt[hp][kt // 4][0:64,
                                                 (kt % 4) * 128:
                                                 (kt % 4) * 128 + 128],
                                qT[hp][qb][0:64, c0:QW],
                                start=True, stop=True,
                                tile_position=(0, 0))
                            nc.tensor.matmul(
                                ps2[:, QW:QW + cw],
                                kTt[hp][kt // 4][64:128,
                                                 (kt % 4) * 128:
                                                 (kt % 4) * 128 + 128],
                                qT[hp][qb][64:128, c0:QW],
                                start=True, stop=True,
                                tile_position=(64, 0))
                            prt = wp.tile([128, 2 * QW], MMDT, tag="probs",
                                          name=f"pr_{hp}_{qb}_{kt}")
                            nc.scalar.activation(
                                prt[:].rearrange(
                                    "p (b j) -> p b j", b=2)[:, :, 0:cw],
                                ps2[:].rearrange(
                                    "p (b j) -> p b j", b=2)[:, :, 0:cw],
                                Exp, scale=0.125)
                            if kt >= nsk - 4:
                                for pv in (prt[:, 0:cw],
                                           prt[:, QW:QW + cw]):
                                    nc.gpsimd.affine_select(
                                        out=pv, in_=pv,
                                        pattern=[[1, cw]], base=0,
                                        channel_multiplier=-1,
                                        compare_op=mybir.AluOpType.is_ge,
                                        fill=0.0)
                            # one filler matmul covers the exp latency before
                            # this step's PV; the rest follow the PV
                            step += 1
                            want = (len(fillers) * step) // steps_total
                            if fi < want:
                                fillers[fi]()
                                fi += 1
                            pv_pair(kt, c0, cw, prt)
                            while fi < want:
                                fillers[fi]()
                                fi += 1
                        for h, po in ((hA, poA), (hB, poB)):
                            den_s = sp.tile([64, QW], f32, tag="dens")
                            nc.vector.tensor_copy(den_s[:], po[64:128, :])
                            rcb = sp.tile([64, QW], f32, tag="recb")
                            nc.vector.reciprocal_approx_fast(
                                out=rcb[:], in_=den_s[:])
                            nc.vector.tensor_tensor(
                                out=attnT[hp][qb][(h % 2) * 64:
                                                  (h % 2) * 64 + 64, :],
                                in0=po[0:HD, :], in1=rcb[:],
                                op=mybir.AluOpType.mult)
                    while fi < len(fillers):
                        fillers[fi]()
                        fi += 1

                # sb=0 projections are a dense block (nothing to hide behind)
                for ch in proj_chunks(0):
                    ch()
                for sb in range(QB):
                    steps = 2 * (sb + 1) * 4
                    outs = []
                    if sb > 0:
                        for st in range((sb - 1) * 4, sb * 4):
                            outs += out_chunks(st)
                    rest = []
                    projs = []
                    if sb + 1 < QB:
                        pch = proj_chunks(sb + 1)
                        # cap interleave rate so deferred DVE work cannot
                        # back up behind the PE racing through fillers
                        ncap = max(0, 2 * steps - len(outs))
                        projs = pch[:ncap]
                        rest = pch[ncap:]
                    emit_attention(sb, merge_prop(projs, outs))
                    for ch in rest:
                        ch()
                for st in range((QB - 1) * 4, QB * 4):
                    for ch in out_chunks(st):
                        ch()

    nc.compile()
    return nc


def _shard_inputs(x, cos, sin, wq, wk, wv, wo):
    x16 = np.asarray(x, dtype=np.float16)
    cosT = np.asarray(cos, np.float32).reshape(S, HD).T
    sinT = np.asarray(sin, np.float32).reshape(S, HD).T
    cs2 = np.ascontiguousarray(
        np.concatenate([cosT, cosT], axis=0).astype(np.float16))
    sn2 = np.ascontiguousarray(
        np.concatenate([sinT, sinT], axis=0).astype(np.float16))

    def pack_w(w, rows):
        # [128, kt*GC + c] = w.T[kt*128 + p, c] for the row-slice of w
        wT = np.asarray(w, np.float16)[rows, :].T          # [D, GC]
        return np.ascontiguousarray(
            wT.reshape(KT, 128, GC).transpose(1, 0, 2).reshape(128, KT * GC))

    def pack_x(xbm):
        # [cb*128 + p, kt*QW + j] = x.T[kt*128 + p, cb*QW + j]
        xT = xbm.T                                         # [D, S]
        return np.ascontiguousarray(
            xT.reshape(KT, 128, QB, QW).transpose(2, 1, 0, 3)
              .reshape(QB * 128, KT * QW))

    in_maps = []
    for c in range(NCORES):
        b, g = c // GROUPS, c % GROUPS
        rows = slice(g * GC, (g + 1) * GC)
        in_maps.append({
            "xB": pack_x(x16[b]),
            "wqs": pack_w(wq, rows),
            "wks": pack_w(wk, rows),
            "wvs": pack_w(wv, rows),
            "woT": np.ascontiguousarray(
                np.asarray(wo, np.float16)[:, rows].T),
            "cs2": cs2,
            "sn2": sn2,
        })
    return in_maps


def _run(inputs, trace=False, trace_kwargs=None):
    if "nc" not in _cache:
        _cache["nc"] = _build()
    nc = _cache["nc"]
    in_maps = _shard_inputs(
        inputs["x"], inputs["cos"], inputs["sin"],
        inputs["wq"], inputs["wk"], inputs["wv"], inputs["wo"])
    res = run_bass_kernel_spmd(
        nc, in_maps, list(range(NCORES)), trace=trace,
        **(trace_kwargs or {}))
    full = np.zeros((B, S, D), dtype=np.float32)
    for c in range(NCORES):
        full[c // GROUPS] += res.results[c]["out"].astype(np.float32)
    return full, res


def kernel(**inputs):
    full, _ = _run(inputs, trace=False)
    return full


# revision 19
# speedup vs baseline: 1.3632x; 1.0050x over previous
"""Trainium2 Bass kernel for nn_Attention_84473416778449.

Reference computation (B=2, S=2048, D=1024, H=16, HD=64, fp32):
    q/k/v = x @ w{q,k,v}.T ; RoPE(q, k) ; causal softmax attention ; out @ wo.T

Sharding: 8 cores = (batch 2) x (head-group 4). Each core computes 4 heads of
one batch end-to-end and a partial output projection over its 256 channels;
the host sums the 4 partials per batch.

v5 structure (evidence-driven through four traced iterations):
  - fp16 host-packed inputs; x blocks load as single dma descriptors.
  - Loads spread over the three DMA-capable queues (sync/scalar/gpsimd),
    ordered by first-need so the PE never bubbles after the ramp
    (per-queue dma bandwidth measured ~115GB/s).
  - PE warm-up matmuls bridge the framework preamble so the HAM clock gate
    reaches 8/8 before real work.
  - Score matmuls for a head pair go to disjoint PE row groups
    (tile_position (0,0)/(64,0)) and run concurrently (measured 4ns apart);
    both write halves of one 2-bank psum tile so ONE batched exp covers the
    pair (halves the ACT per-instruction overhead).
  - V tiles carry 64 replicated ones-columns per head, so the PV matmul
    emits the softmax denominator replicated across psum rows 64:128 and
    normalization is just reciprocal+multiply on the DVE (no psum row copy,
    no gpsimd partition broadcast on the critical path).
  - The RoPE matmul reuses its own chain's psA tile (dead after the cos/sin
    multiplies) and is emitted a few chunks late so it never head-of-line
    blocks the PE queue.
"""
import sys

if "/opt/trn_rl_repo" not in sys.path:
    sys.path.insert(0, "/opt/trn_rl_repo")

import numpy as np

import concourse.bass as bass
import concourse.mybir as mybir
import concourse.tile as tile
from concourse import bacc
from concourse.bass_utils import run_bass_kernel_spmd

B, S, D, H, HD = 2, 2048, 1024, 16, 64
NCORES = 8
GROUPS = 4            # head groups
GH = H // GROUPS      # heads per group = 4
GC = GH * HD          # channels per group = 256
KT = D // 128         # 8 k-tiles over D
ST = S // 128         # 16 s-tiles
QB = 4                # sq blocks of 512
QW = S // QB          # 512
XW = KT * QW          # 4096: packed x block width

f32 = mybir.dt.float32
MMDT = mybir.dt.float16   # matmul-operand dtype
Exp = mybir.ActivationFunctionType.Exp
Copy = mybir.ActivationFunctionType.Copy

_cache = {}


def _build():
    nc = bacc.Bacc("TRN2", num_devices=NCORES)

    # host-packed: row-block cb holds [p, kt*QW + j] = x[b].T[kt*128+p, cb*QW+j]
    xB = nc.dram_tensor("xB", [QB * 128, XW], MMDT, kind="ExternalInput").ap()
    # host-packed: [p, kt*GC + c] = w.T[kt*128 + p, c]
    wqs = nc.dram_tensor("wqs", [128, KT * GC], MMDT, kind="ExternalInput").ap()
    wks = nc.dram_tensor("wks", [128, KT * GC], MMDT, kind="ExternalInput").ap()
    wvs = nc.dram_tensor("wvs", [128, KT * GC], MMDT, kind="ExternalInput").ap()
    woT = nc.dram_tensor("woT", [GC, D], MMDT, kind="ExternalInput").ap()
    cs2 = nc.dram_tensor("cs2", [128, S], MMDT, kind="ExternalInput").ap()
    sn2 = nc.dram_tensor("sn2", [128, S], MMDT, kind="ExternalInput").ap()
    out = nc.dram_tensor("out", [S, D], MMDT, kind="ExternalOutput").ap()

    with tile.TileContext(nc) as tc:
        with tc.tile_pool(name="persist", bufs=1) as pp, \
             tc.tile_pool(name="rope", bufs=3) as rp, \
             tc.tile_pool(name="probs", bufs=6) as wp, \
             tc.tile_pool(name="outsb", bufs=2) as op_, \
             tc.tile_pool(name="small", bufs=3) as sp:

            xb = [pp.tile([128, XW], MMDT, tag=f"xb{cb}", name=f"xb{cb}")
                  for cb in range(QB)]

            def xs(kt, cb):
                return xb[cb][:, kt * QW:(kt + 1) * QW]

            def load_w(src, eng):
                t = pp.tile([128, KT * GC], MMDT, tag=f"w{src.tensor.name}",
                            name=f"w{src.tensor.name}")
                eng.dma_start(t[:], src)
                return t

            # ---- loads spread across the three DMA queues by first-need ----
            warm = pp.tile([128, 128], MMDT, tag="warm")
            nc.gpsimd.memset(warm[:], 0.0)
            warm2 = pp.tile([128, QW + 128], MMDT, tag="warm2")
            nc.gpsimd.memset(warm2[:], 0.0)
            nc.gpsimd.dma_start(xb[0][:, 0:XW // 2], xB[0:128, 0:XW // 2])
            wq_s = load_w(wqs, nc.sync)
            nc.scalar.dma_start(xb[0][:, XW // 2:XW], xB[0:128, XW // 2:XW])
            cs_sb = pp.tile([128, S], MMDT, tag="cs")
            nc.sync.dma_start(cs_sb[:], cs2[:])
            sn_sb = pp.tile([128, S], MMDT, tag="sn")
            nc.scalar.dma_start(sn_sb[:], sn2[:])
            wk_s = load_w(wks, nc.gpsimd)
            wv_s = load_w(wvs, nc.scalar)
            wo_s = []
            for kt in range(2):
                t = pp.tile([128, D], MMDT, tag=f"wo{kt}", name=f"wo{kt}")
                nc.sync.dma_start(t[:], woT[kt * 128:(kt + 1) * 128, :])
                wo_s.append(t)
            nc.gpsimd.dma_start(xb[1][:], xB[128:256, :])
            nc.gpsimd.dma_start(xb[2][:], xB[256:384, :])
            nc.gpsimd.dma_start(xb[3][:], xB[384:512, :])

            # ---- PE warm-up: dummy matmuls while the first DMAs land ------
            with tc.tile_pool(name="psW", bufs=1, space="PSUM") as psW:
                wps = psW.tile([128, QW], f32, tag="wps")
                for _ in range(20):
                    nc.tensor.matmul(wps[:, 0:128], warm[:], warm[:],
                                     start=True, stop=True)
                for _ in range(26):
                    nc.tensor.matmul(wps[:], warm2[:, QW:QW + 128],
                                     warm2[:, 0:QW],
                                     start=True, stop=True)

            # ---- constants -------------------------------------------------
            cscratch = pp.tile([128, 128], f32, tag="cscratch")
            nc.gpsimd.memset(cscratch[:], 0.0)
            for blk in range(2):
                sub = cscratch[blk * 64:(blk + 1) * 64,
                               blk * 64:(blk + 1) * 64]
                nc.gpsimd.affine_select(   # -1 where p - f == 32
                    out=sub, in_=sub, pattern=[[-1, 64]], base=-32,
                    channel_multiplier=1,
                    compare_op=mybir.AluOpType.not_equal, fill=-1.0)
                nc.gpsimd.affine_select(   # +1 where f - p == 32
                    out=sub, in_=sub, pattern=[[1, 64]], base=-32,
                    channel_multiplier=-1,
                    compare_op=mybir.AluOpType.not_equal, fill=1.0)
            rt2 = pp.tile([128, 128], MMDT, tag="rt2")
            nc.vector.tensor_copy(rt2[:], cscratch[:])

            qT = [[pp.tile([128, QW], MMDT, tag=f"qT{i}_{b}",
                           name=f"qT{i}_{b}") for b in range(QB)]
                  for i in range(2)]
            kTt = [[pp.tile([128, QW], MMDT, tag=f"kT{i}_{b}",
                            name=f"kT{i}_{b}") for b in range(QB)]
                   for i in range(2)]
            attnT = [[pp.tile([128, QW], MMDT, tag=f"aT{i}_{b}",
                              name=f"aT{i}_{b}") for b in range(QB)]
                     for i in range(2)]
            # per head h: cols h*128+0:64 = v payload, h*128+64:128 = ones
            # (PV then emits the denominator replicated on psum rows 64:128)
            v_sb = [pp.tile([128, 4 * 128], MMDT, tag=f"v{i}",
                            name=f"v{i}") for i in range(ST)]
            ones256 = pp.tile([128, 256], f32, tag="ones256")
            nc.gpsimd.memset(ones256[:], 1.0)
            for vt in v_sb:
                nc.vector.tensor_copy(
                    vt[:].rearrange("p (h e) -> p h e", e=128)[:, :, HD:128],
                    ones256[:].rearrange("p (h e) -> p h e", e=HD))

            with tc.tile_pool(name="psA", bufs=2, space="PSUM") as psA, \
                 tc.tile_pool(name="psS", bufs=2, space="PSUM") as psS, \
                 tc.tile_pool(name="psO", bufs=2, space="PSUM") as psO:

                def qk_chain_chunks(w_src, dst, hp, sb):
                    """[mm x8, fin_dve] plus a separate fin_rope chunk that the
                    caller emits a few chunks later (it holds the PE rope
                    matmul, which must not wait at the head of the PE queue).
                    The rope matmul reuses the chain's own psA tile."""
                    cols = slice(sb * QW, (sb + 1) * QW)
                    box = {}

                    def mk_mm(kt):
                        def f():
                            if kt == 0:
                                box['pq'] = psA.tile(
                                    [128, QW], f32, tag="proj",
                                    name=f"pq{hp}_{sb}")
                            nc.tensor.matmul(
                                box['pq'][:],
                                w_src[:, kt * GC + hp * 128:
                                      kt * GC + hp * 128 + 128],
                                xs(kt, sb),
                                start=(kt == 0), stop=(kt == KT - 1))
                        return f

                    def fin_dve():
                        pq = box['pq']
                        box['tcs'] = rp.tile([128, QW], MMDT, tag="tcs",
                                             name=f"tcs{hp}_{sb}")
                        nc.vector.tensor_tensor(
                            out=box['tcs'][:], in0=pq[:], in1=cs_sb[:, cols],
                            op=mybir.AluOpType.mult)
                        box['tsn'] = rp.tile([128, QW], MMDT, tag="tsn",
                                             name=f"tsn{hp}_{sb}")
                        nc.vector.tensor_tensor(
                            out=box['tsn'][:], in0=pq[:], in1=sn_sb[:, cols],
                            op=mybir.AluOpType.mult)

                    def fin_rope():
                        pq = box['pq']
                        nc.tensor.matmul(pq[:], rt2[:], box['tsn'][:],
                                         start=True, stop=True)
                        nc.vector.tensor_tensor(
                            out=dst[hp][sb][:], in0=pq[:], in1=box['tcs'][:],
                            op=mybir.AluOpType.add)
                    return [mk_mm(kt) for kt in range(KT)] + [fin_dve], fin_rope

                def v_chain_chunks(st):
                    box = {}

                    def mk_mm(kt):
                        def f():
                            if kt == 0:
                                box['pv'] = psA.tile(
                                    [128, QW], f32, tag="proj",
                                    name=f"pv_{st}")
                            nc.tensor.matmul(
                                box['pv'][:, 0:GC],
                                xs(kt, st // 4)[:, (st % 4) * 128:
                                                (st % 4) * 128 + 128],
                                wv_s[:, kt * GC:(kt + 1) * GC],
                                start=(kt == 0), stop=(kt == KT - 1))
                        return f

                    def fin():
                        vhe = v_sb[st][:].rearrange("p (h e) -> p h e", e=128)
                        nc.scalar.activation(
                            vhe[:, :, 0:HD],
                            box['pv'][:, 0:GC].rearrange(
                                "p (h d) -> p h d", d=HD),
                            Copy)
                    return [mk_mm(kt) for kt in range(KT)] + [fin]

                def proj_chunks(sbx):
                    """Chunk list for block sbx with each chain's rope matmul
                    deferred into the following chain's matmul stream."""
                    chunks = []
                    pending = []

                    def flush():
                        while pending:
                            chunks.append(pending.pop(0))

                    for hp in range(2):
                        for w_src, dst in ((wq_s, qT), (wk_s, kTt)):
                            main, fin_rope = qk_chain_chunks(
                                w_src, dst, hp, sbx)
                            chunks += main[:3]
                            flush()
                            chunks += main[3:]
                            pending.append(fin_rope)
                    for st in range(sbx * 4, sbx * 4 + 4):
                        vc = v_chain_chunks(st)
                        chunks += vc[:3]
                        flush()
                        chunks += vc[3:]
                    return chunks

                def out_chunks(st, on_dve=False):
                    box = {}

                    def mms(db):
                        def f():
                            box[db] = psA.tile([128, QW], f32, tag="proj",
                                               name=f"pc_{st}_{db}")
                            if db == 0:
                                box['ob'] = op_.tile([128, 2 * QW], MMDT,
                                                     tag="outsb",
                                                     name=f"ob_{st}")
                            for kt in range(2):
                                nc.tensor.matmul(
                                    box[db][:],
                                    attnT[kt][st // 4][:, (st % 4) * 128:
                                                       (st % 4) * 128 + 128],
                                    wo_s[kt][:, db * QW:(db + 1) * QW],
                                    start=(kt == 0), stop=(kt == 1))
                        return f

                    def cp(db):
                        def f():
                            if on_dve:
                                nc.vector.tensor_copy(
                                    box['ob'][:, db * QW:(db + 1) * QW],
                                    box[db][:])
                            else:
                                nc.scalar.activation(
                                    box['ob'][:, db * QW:(db + 1) * QW],
                                    box[db][:], Copy)
                            if db == 1:
                                nc.sync.dma_start(
                                    out[st * 128:(st + 1) * 128, :],
                                    box['ob'][:])
                        return f
                    return [mms(0), cp(0), mms(1), cp(1)]

                def merge_prop(a, b):
                    res, ia, ib = [], 0, 0
                    la, lb = max(len(a), 1), max(len(b), 1)
                    while ia < len(a) or ib < len(b):
                        if ib >= len(b) or (ia < len(a) and
                                            ia * lb <= ib * la):
                            res.append(a[ia]); ia += 1
                        else:
                            res.append(b[ib]); ib += 1
                    return res

                def emit_attention(qb, fillers):
                    nsk = (qb + 1) * 4
                    steps_total = 2 * nsk
                    step = 0
                    fi = 0
                    for hp in range(2):
                        hA, hB = 2 * hp, 2 * hp + 1
                        poA = psO.tile([128, QW], f32, tag="pvacc",
                                       name=f"poA_{hp}_{qb}")
                        poB = psO.tile([128, QW], f32, tag="pvacc",
                                       name=f"poB_{hp}_{qb}")

                        def pv_pair(kt, c0, cw, prt):
                            nc.tensor.matmul(
                                poA[:, c0:QW],
                                v_sb[kt][:, hA * 128:hA * 128 + 128],
                                prt[:, 0:cw],
                                start=(kt == 0), stop=(kt == nsk - 1))
                            nc.tensor.matmul(
                                poB[:, c0:QW],
                                v_sb[kt][:, hB * 128:hB * 128 + 128],
                                prt[:, QW:QW + cw],
                                start=(kt == 0), stop=(kt == nsk - 1))

                        for kt in range(nsk):
                            c0 = max(0, kt * 128 - qb * QW)
                            cw = QW - c0
                            ps2 = psS.tile([128, 2 * QW], f32, tag="score",
                                           name=f"sc_{hp}_{qb}_{kt}")
                            nc.tensor.matmul(
                                ps2[:, 0:cw],
                                kTt[hp][kt // 4][0:64,
                                                 (kt % 4) * 128:
                                                 (kt % 4) * 128 + 128],
                                qT[hp][qb][0:64, c0:QW],
                                start=True, stop=True,
                                tile_position=(0, 0))
                            nc.tensor.matmul(
                                ps2[:, QW:QW + cw],
                                kTt[hp][kt // 4][64:128,
                                                 (kt % 4) * 128:
                                                 (kt % 4) * 128 + 128],
                                qT[hp][qb][64:128, c0:QW],
                                start=True, stop=True,
                                tile_position=(64, 0))
                            prt = wp.tile([128, 2 * QW], MMDT, tag="probs",
                                          name=f"pr_{hp}_{qb}_{kt}")
                            nc.scalar.activation(
                                prt[:].rearrange(
                                    "p (b j) -> p b j", b=2)[:, :, 0:cw],
                                ps2[:].rearrange(
                                    "p (b j) -> p b j", b=2)[:, :, 0:cw],
                                Exp, scale=0.125)
                            if kt >= nsk - 4:
                                for pv in (prt[:, 0:cw],
                                           prt[:, QW:QW + cw]):
                                    nc.gpsimd.affine_select(
                                        out=pv, in_=pv,
                                        pattern=[[1, cw]], base=0,
                                        channel_multiplier=-1,
                                        compare_op=mybir.AluOpType.is_ge,
                                        fill=0.0)
                            # one filler matmul covers the exp latency before
                            # this step's PV; the rest follow the PV
                            step += 1
                            want = (len(fillers) * step) // steps_total
                            if fi < want:
                                fillers[fi]()
                                fi += 1
                            pv_pair(kt, c0, cw, prt)
                            while fi < want:
                                fillers[fi]()
                                fi += 1
                        for h, po in ((hA, poA), (hB, poB)):
                            den_s = sp.tile([64, QW], f32, tag="dens")
                            nc.vector.tensor_copy(den_s[:], po[64:128, :])
                            rcb = sp.tile([64, QW], f32, tag="recb")
                            nc.vector.reciprocal_approx_fast(
                                out=rcb[:], in_=den_s[:])
                            nc.vector.tensor_tensor(
                                out=attnT[hp][qb][(h % 2) * 64:
                                                  (h % 2) * 64 + 64, :],
                                in0=po[0:HD, :], in1=rcb[:],
                                op=mybir.AluOpType.mult)
                    while fi < len(fillers):
                        fillers[fi]()
                        fi += 1

                # sb=0 projections are a dense block (nothing to hide behind)
                for ch in proj_chunks(0):
                    ch()
                for sb in range(QB):
                    steps = 2 * (sb + 1) * 4
                    outs = []
                    if sb > 0:
                        for st in range((sb - 1) * 4, sb * 4):
                            outs += out_chunks(st)
                    rest = []
                    projs = []
                    if sb + 1 < QB:
                        pch = proj_chunks(sb + 1)
                        # cap interleave rate so deferred DVE work cannot
                        # back up behind the PE racing through fillers
                        ncap = max(0, 2 * steps - len(outs))
                        projs = pch[:ncap]
                        rest = pch[ncap:]
                    emit_attention(sb, merge_prop(projs, outs))
                    for ch in rest:
                        ch()
                for st in range((QB - 1) * 4, QB * 4):
                    for ch in out_chunks(st, on_dve=True):
                        ch()

    nc.compile()
    return nc


def _shard_inputs(x, cos, sin, wq, wk, wv, wo):
    x16 = np.asarray(x, dtype=np.float16)
    cosT = np.asarray(cos, np.float32).reshape(S, HD).T
    sinT = np.asarray(sin, np.float32).reshape(S, HD).T
    cs2 = np.ascontiguousarray(
        np.concatenate([cosT, cosT], axis=0).astype(np.float16))
    sn2 = np.ascontiguousarray(
        np.concatenate([sinT, sinT], axis=0).astype(np.float16))

    def pack_w(w, rows):
        # [128, kt*GC + c] = w.T[kt*128 + p, c] for the row-slice of w
        wT = np.asarray(w, np.float16)[rows, :].T          # [D, GC]
        return np.ascontiguousarray(
            wT.reshape(KT, 128, GC).transpose(1, 0, 2).reshape(128, KT * GC))

    def pack_x(xbm):
        # [cb*128 + p, kt*QW + j] = x.T[kt*128 + p, cb*QW + j]
        xT = xbm.T                                         # [D, S]
        return np.ascontiguousarray(
            xT.reshape(KT, 128, QB, QW).transpose(2, 1, 0, 3)
              .reshape(QB * 128, KT * QW))

    in_maps = []
    for c in range(NCORES):
        b, g = c // GROUPS, c % GROUPS
        rows = slice(g * GC, (g + 1) * GC)
        in_maps.append({
            "xB": pack_x(x16[b]),
            "wqs": pack_w(wq, rows),
            "wks": pack_w(wk, rows),
            "wvs": pack_w(wv, rows),
            "woT": np.ascontiguousarray(
                np.asarray(wo, np.float16)[:, rows].T),
            "cs2": cs2,
            "sn2": sn2,
        })
    return in_maps


def _run(inputs, trace=False, trace_kwargs=None):
    if "nc" not in _cache:
        _cache["nc"] = _build()
    nc = _cache["nc"]
    in_maps = _shard_inputs(
        inputs["x"], inputs["cos"], inputs["sin"],
        inputs["wq"], inputs["wk"], inputs["wv"], inputs["wo"])
    res = run_bass_kernel_spmd(
        nc, in_maps, list(range(NCORES)), trace=trace,
        **(trace_kwargs or {}))
    full = np.zeros((B, S, D), dtype=np.float32)
    for c in range(NCORES):
        full[c // GROUPS] += res.results[c]["out"].astype(np.float32)
    return full, res


def kernel(**inputs):
    full, _ = _run(inputs, trace=False)
    return full
